# revision 1
# baseline (speedup 1.0000x reference)
"""Trainium2 Bass kernel for nn_Bdfdv_51170240364850 (gnn_message_passing).

Computes, for mode pairs (il, im) with im <= il (L1 = 5 modes each way) and
spatial/velocity grid (nx=1024, nv=512):

  D[il,im] = base + (-1j)*im*bx*F[il,im] + cB*bm*F[il,im+1]
             + [im==0] Re(cC*bp*F[il,1])
  base     = 0.5*bm*F[il,im-1]  (il>=1, 1<=im<=il)   else  D0[il,im]

with bx = b[:,0], bm = b[:,1]+1j b[:,2], bp = conj(bm),
cB = -(il-im)(il+im+1)/2, cC = -il(il+1).

Strategy: pure data-parallel over nx across 8 NeuronCores (nx=128 per core,
mapped onto the 128 SBUF partitions). All per-x scalar coefficient products
(constants x b-columns) are precomputed host-side into a small [128, 48]
table, so every device-side term is a single fused
scalar_tensor_tensor(out = in0 * scal + in1) instruction. Mode (il,0) folds
the term4 contribution into the cB coefficients (cC = 2*cB(il,0)).

Inputs are packed host-side into one [128, CIN] f32 array per core (valid
mode slices only: (0,0) passes through host-side); output is one
[128, COUT] f32 array per core, unpacked host-side into the complex64
(5,5,1024,512) result.
"""

import numpy as np

import bass_rust
import concourse.bass as bass
import concourse.tile as tile
from concourse import mybir
from concourse.bass_utils import run_bass_kernel_spmd
from concourse.vector_clock import ScopedClock

L1 = 5
NX = 1024
NV = 512
NCORES = 8
XS = NX // NCORES  # 128, = SBUF partitions

F32 = mybir.dt.float32

# ---------------------------------------------------------------------------
# scal table columns
H1, H2, NH2 = 0, 1, 2          # 0.5*b1, 0.5*b2, -0.5*b2


def A_P(m):                    # +m*b0  (m = 1..4)
    return 3 + (m - 1)


def A_N(m):                    # -m*b0
    return 7 + (m - 1)


def Q1(il):                    # 3*cB0*b1, cB0 = -il(il+1)/2
    return 11 + (il - 1)


def Q2(il):                    # cB0*b2
    return 15 + (il - 1)


def R1(il):                    # cB0*b1
    return 19 + (il - 1)


CB_PAIRS = [(2, 1), (3, 1), (3, 2), (4, 1), (4, 2), (4, 3)]


def CB1(il, im):               # cB*b1
    return 23 + 3 * CB_PAIRS.index((il, im))


def CB2(il, im):               # cB*b2
    return 24 + 3 * CB_PAIRS.index((il, im))


def NCB2(il, im):              # -cB*b2
    return 25 + 3 * CB_PAIRS.index((il, im))


NSCAL = 48  # 41 used, padded

# packed input layout: [scal (NSCAL) | row blocks il=1..4]
# row block: Fr slots (il+1), Fi slots (il+1), D0r, D0i  -- each slot NV cols
IN_OFF = {}
_o = NSCAL
for _il in range(1, L1):
    IN_OFF[_il] = _o
    _o += (2 * (_il + 1) + 2) * NV
CIN = _o

# packed output layout: row blocks il=1..4, each: Dr slots (il+1), Di slots
OUT_OFF = {}
_o = 0
for _il in range(1, L1):
    OUT_OFF[_il] = _o
    _o += 2 * (_il + 1) * NV
COUT = _o


def _cB(il, im):
    return -(il - im) * (il + im + 1) / 2.0


def build_scal(b_sh):
    """b_sh: [XS, 3] float32 -> [XS, NSCAL] float32 coefficient table."""
    b0, b1, b2 = b_sh[:, 0], b_sh[:, 1], b_sh[:, 2]
    s = np.zeros((XS, NSCAL), np.float32)
    s[:, H1] = 0.5 * b1
    s[:, H2] = 0.5 * b2
    s[:, NH2] = -0.5 * b2
    for m in range(1, L1):
        s[:, A_P(m)] = m * b0
        s[:, A_N(m)] = -m * b0
    for il in range(1, L1):
        cB0 = _cB(il, 0)
        s[:, Q1(il)] = 3.0 * cB0 * b1
        s[:, Q2(il)] = cB0 * b2
        s[:, R1(il)] = cB0 * b1
    for (il, im) in CB_PAIRS:
        cB = _cB(il, im)
        s[:, CB1(il, im)] = cB * b1
        s[:, CB2(il, im)] = cB * b2
        s[:, NCB2(il, im)] = -cB * b2
    return s


# ---------------------------------------------------------------------------
# The walrus build in this container rejects instructions carrying more than
# ONE sync-wait ("Too many sync wait commands", setupSyncWait in
# CoreV2/V3GenImpl). Tile's scheduler routinely attaches several. Post-pass:
# hoist all but the last wait of each instruction onto same-engine NOPs
# inserted immediately before it (same basic block, so per-engine program
# order is preserved).
def split_multiwaits(nc):
    for f in nc.m.functions:
        for blk in f.blocks:
            new = []
            changed = False
            for ins in blk.instructions:
                si = ins.sync_info
                if si is not None and len(si.on_wait) > 1:
                    waits = list(si.on_wait)
                    for w in waits[:-1]:
                        nop = mybir.InstNoOp(
                            name=nc.get_next_instruction_name(),
                            engine=ins.engine,
                            bass_nofuse=True,
                            sync_info=mybir.SyncInfo(on_wait=[w],
                                                     on_update=[]),
                        )
                        new.append(nop)
                    ins.sync_info = bass_rust.SyncInfo(
                        on_wait=[waits[-1]], on_update=list(si.on_update))
                    changed = True
                new.append(ins)
            if changed:
                blk.instructions = new


# ---------------------------------------------------------------------------
def _pair(ap, step_elems, nblocks=2):
    """Turn a contiguous [P, L] AP into [P, nblocks, L] with the given
    element step between blocks (may be negative)."""
    c = ap.copy()
    v = c.ap
    last = v.pop()
    v.append((step_elems, nblocks))
    v.append(tuple(last))
    c.ap = v
    return c


def build_bass(split=True):
    """Pair-merged elementwise kernel.

    Per il-row SBUF layout: in-tile [nfi | fr | fi] (ns slots each,
    ns = il+1), out-tile [dr | di]. nfi = -fi (one ACT negate per row) lets
    every b2-coefficient op run as ONE fused scalar_tensor_tensor covering
    BOTH the Dr and Di halves (2-block strided APs, same per-x scalar
    column), halving DVE instruction count. ACT produces the set-term heads
    (b1 pair) and the negates; DVE runs all fused accumulates. Input DMAs
    are chained so row 1 lands first and compute ramps early.
    """
    from bass_rust import add_dep_helper

    MULT = mybir.AluOpType.mult
    ADD = mybir.AluOpType.add

    nc = bass.Bass()
    pin = nc.dram_tensor("pin", [XS, CIN], F32, kind="ExternalInput").ap()
    pout = nc.dram_tensor("pout", [XS, COUT], F32, kind="ExternalOutput").ap()

    with tile.TileContext(nc) as tc:
        with tc.tile_pool(name="m", bufs=1) as pool:
            scal = pool.tile([XS, NSCAL], F32, tag="scal")
            prev_dma = nc.sync.dma_start(scal[:], pin[:, 0:NSCAL])

            def sc(col):
                return scal[:, col:col + 1]

            def chain(d):
                nonlocal prev_dma
                add_dep_helper(d.ins, prev_dma.ins,
                               reason="serialize input DMAs")
                prev_dma = d

            row_in = {}
            row_d0 = {}
            for il in range(1, L1):
                ns = il + 1
                t = pool.tile([XS, 3 * ns * NV], F32, tag=f"in{il}")
                d0 = pool.tile([XS, 2 * NV], F32, tag=f"d0_{il}")
                # pin row block: fr slots, fi slots, d0r, d0i
                o = IN_OFF[il]
                chain(nc.sync.dma_start(t[:, ns * NV:3 * ns * NV],
                                        pin[:, o:o + 2 * ns * NV]))
                chain(nc.sync.dma_start(
                    d0[:], pin[:, o + 2 * ns * NV:o + (2 * ns + 2) * NV]))
                row_in[il] = t
                row_d0[il] = d0

            for il in range(1, L1):
                t = row_in[il]
                d0 = row_d0[il]
                ns = il + 1
                nfi = t[:, 0:ns * NV]
                fr = t[:, ns * NV:2 * ns * NV]
                fi = t[:, 2 * ns * NV:3 * ns * NV]
                d0r = d0[:, 0:NV]
                d0i = d0[:, NV:2 * NV]

                to = pool.tile([XS, 2 * ns * NV], F32, tag=f"out{il}")
                dr = to[:, 0:ns * NV]
                di = to[:, ns * NV:2 * ns * NV]

                S = ns * NV  # slot-block stride (elements)

                def sl(buf, k, n=1):
                    return buf[:, k * NV:(k + n) * NV]

                # ACT: nfi = -fi
                nc.scalar.mul(nfi, fi, -1.0)
                # ACT set-b1 head pair:
                #   dr[1..il] = 0.5*b1*fr[0..il-1]; di[1..il] = 0.5*b1*fi[..]
                nc.scalar.mul(_pair(sl(dr, 1, il), S), _pair(sl(fr, 0, il), S),
                              sc(H1))
                # DVE set-b2 pair: dr += 0.5*b2*nfi[0..il-1];
                #                  di += 0.5*b2*fr[0..il-1]
                nc.vector.scalar_tensor_tensor(
                    _pair(sl(dr, 1, il), S), _pair(sl(nfi, 0, il), S),
                    sc(H2), _pair(sl(dr, 1, il), S), MULT, ADD)
                # im=0 bases (distinct b1 scalars -> two singles)
                nc.vector.scalar_tensor_tensor(
                    sl(dr, 0), sl(fr, 1), sc(Q1(il)), d0r, MULT, ADD)
                nc.vector.scalar_tensor_tensor(
                    sl(di, 0), sl(fi, 1), sc(R1(il)), d0i, MULT, ADD)
                # im=0 b2 terms (no negative-stride pairing on HW)
                nc.vector.scalar_tensor_tensor(
                    sl(dr, 0), sl(fi, 1), sc(Q2(il)), sl(dr, 0), MULT, ADD)
                nc.vector.scalar_tensor_tensor(
                    sl(di, 0), sl(fr, 1), sc(Q2(il)), sl(di, 0), MULT, ADD)
                # cB pairs (im=1..il-1)
                for im in range(1, il):
                    nc.vector.scalar_tensor_tensor(
                        _pair(sl(dr, im), S), _pair(sl(fr, im + 1), S),
                        sc(CB1(il, im)), _pair(sl(dr, im), S), MULT, ADD)
                    nc.vector.scalar_tensor_tensor(
                        _pair(sl(dr, im), S), _pair(sl(nfi, im + 1), S),
                        sc(CB2(il, im)), _pair(sl(dr, im), S), MULT, ADD)
                # cA singles
                for im in range(1, il + 1):
                    nc.vector.scalar_tensor_tensor(
                        sl(dr, im), sl(fi, im), sc(A_P(im)), sl(dr, im),
                        MULT, ADD)
                    nc.vector.scalar_tensor_tensor(
                        sl(di, im), sl(fr, im), sc(A_N(im)), sl(di, im),
                        MULT, ADD)

                nc.sync.dma_start(
                    pout[:, OUT_OFF[il]:OUT_OFF[il] + ns * NV], dr)
                nc.sync.dma_start(
                    pout[:, OUT_OFF[il] + ns * NV:OUT_OFF[il] + 2 * ns * NV],
                    di)

    if split:
        split_multiwaits(nc)
    return nc


# ---------------------------------------------------------------------------
def pack_inputs(prev_f_re, prev_f_im, delta0_re, delta0_im, b):
    """-> list of per-core {'pin': [XS, CIN] f32}."""
    in_maps = []
    for c in range(NCORES):
        X = slice(c * XS, (c + 1) * XS)
        p = np.empty((XS, CIN), np.float32)
        p[:, :NSCAL] = 0.0
        p[:, :NSCAL][:, :41] = build_scal(np.asarray(b[X], np.float32))[:, :41]
        for il in range(1, L1):
            o = IN_OFF[il]
            ns = il + 1
            p[:, o:o + ns * NV] = (
                np.asarray(prev_f_re[il, :ns, X, :], np.float32)
                .transpose(1, 0, 2).reshape(XS, ns * NV))
            o += ns * NV
            p[:, o:o + ns * NV] = (
                np.asarray(prev_f_im[il, :ns, X, :], np.float32)
                .transpose(1, 0, 2).reshape(XS, ns * NV))
            o += ns * NV
            p[:, o:o + NV] = np.asarray(delta0_re[il, 0, X, :], np.float32)
            o += NV
            p[:, o:o + NV] = np.asarray(delta0_im[il, 0, X, :], np.float32)
        in_maps.append({"pin": p})
    return in_maps


def unpack_outputs(results, delta0_re, delta0_im):
    out = np.zeros((L1, L1, NX, NV), np.complex64)
    out[0, 0] = np.asarray(delta0_re[0, 0]) + 1j * np.asarray(delta0_im[0, 0])
    for c in range(NCORES):
        X = slice(c * XS, (c + 1) * XS)
        p = results[c]["pout"]
        for il in range(1, L1):
            o = OUT_OFF[il]
            ns = il + 1
            dr = p[:, o:o + ns * NV].reshape(XS, ns, NV).transpose(1, 0, 2)
            di = (p[:, o + ns * NV:o + 2 * ns * NV]
                  .reshape(XS, ns, NV).transpose(1, 0, 2))
            out[il, :ns, X, :] = dr + 1j * di
    return out


_NC_CACHE = None


def get_nc():
    global _NC_CACHE
    if _NC_CACHE is None:
        _NC_CACHE = build_bass()
    return _NC_CACHE


def kernel(prev_f_re, prev_f_im, delta0_re, delta0_im, b, v):
    in_maps = pack_inputs(prev_f_re, prev_f_im, delta0_re, delta0_im, b)
    res = run_bass_kernel_spmd(get_nc(), in_maps, list(range(NCORES)))
    return unpack_outputs(res.results, delta0_re, delta0_im)



# revision 3
# speedup vs baseline: 1.4385x; 1.4385x over previous
"""Trainium2 Bass kernel for nn_Bdfdv_51170240364850 (gnn_message_passing).

Computes, for mode pairs (il, im) with im <= il (L1 = 5 modes each way) and
grid (nx=1024, nv=512):

  D[il,im] = base + (-1j)*im*bx*F[il,im] + cB*bm*F[il,im+1]
             + [im==0] Re(cC*bp*F[il,1])
  base     = 0.5*bm*F[il,im-1]  (il>=1, 1<=im<=il)   else  D0[il,im]

with bx = b[:,0], bm = b[:,1]+1j b[:,2], bp = conj(bm),
cB = -(il-im)(il+im+1)/2, cC = -il(il+1).

Strategy: pure data-parallel over nx across 8 NeuronCores (nx=128 per core on
the 128 SBUF partitions), bf16 end-to-end (tolerance is 2e-2; bf16 keeps
~4e-3), with ALL per-x scalar products executed on the Tensor engine as
diagonal-weight matmuls accumulating in PSUM:

    out[x, :] += c(x) * in[x, :]   ==   PSUM += diag(c).T @ in

Each of the 28 valid output slots (dr/di per (il, im>=0), row 0 handled on
host) is one PSUM-bank accumulation chain of 3-5 matmuls (N=512 columns
each). The 42 distinct per-x coefficient vectors (products of constants with
b columns, computed host-side into a [128, 48] f32 table) become diagonal
[128,128] bf16 weights built on-device by one tensor_scalar multiply of an
identity tile each. ACT and DVE only drain PSUM banks to bf16 SBUF tiles
(alternating), which DMA out per row. No other elementwise work exists.
"""

import numpy as np
import ml_dtypes

import bass_rust
import concourse.bass as bass
import concourse.tile as tile
from concourse import mybir
from concourse.bass_utils import run_bass_kernel_spmd

L1 = 5
NX = 1024
NV = 512
NCORES = 8
XS = NX // NCORES  # 128 = SBUF partitions

F32 = mybir.dt.float32
BF16 = mybir.dt.bfloat16
NPBF16 = ml_dtypes.bfloat16

# ---------------------------------------------------------------------------
# coefficient column registry (host f32 table -> on-device diagonal weights)
PAIRS = [(2, 1), (3, 1), (3, 2), (4, 1), (4, 2), (4, 3)]  # inner (il, im)


def _cB(il, im):
    return -(il - im) * (il + im + 1) / 2.0


COL = {"one": 0, "h1": 1, "h2p": 2, "h2n": 3}
for _il in range(1, L1):
    COL[("q1", _il)] = 3 + _il           # 4..7
    COL[("r1", _il)] = 7 + _il           # 8..11
    COL[("q2", _il)] = 11 + _il          # 12..15
for _m in range(1, L1):
    COL[("b0p", _m)] = 15 + _m           # 16..19
    COL[("b0n", _m)] = 19 + _m           # 20..23
for _k, _p in enumerate(PAIRS):
    COL[("c1",) + _p] = 24 + 3 * _k
    COL[("c2p",) + _p] = 25 + 3 * _k
    COL[("c2n",) + _p] = 26 + 3 * _k
NCOL = 42
NSCAL = 48  # padded

# packed input layout (bf16): [I (128 cols) | row blocks il=1..4]
# row block: fr slots (ns), fi slots (ns), d0r, d0i  -- each slot NV cols
IN_OFF = {}
_o = 128
for _il in range(1, L1):
    IN_OFF[_il] = _o
    _o += (2 * (_il + 1) + 2) * NV
CIN = _o

# packed output layout (bf16): row blocks il=1..4: dr slots (ns), di slots
OUT_OFF = {}
_o = 0
for _il in range(1, L1):
    OUT_OFF[_il] = _o
    _o += 2 * (_il + 1) * NV
COUT = _o


def build_scal(b_sh):
    """b_sh: [XS, 3] float32 -> [XS, NSCAL] float32 coefficient table."""
    b0, b1, b2 = b_sh[:, 0], b_sh[:, 1], b_sh[:, 2]
    s = np.zeros((XS, NSCAL), np.float32)
    s[:, COL["one"]] = 1.0
    s[:, COL["h1"]] = 0.5 * b1
    s[:, COL["h2p"]] = 0.5 * b2
    s[:, COL["h2n"]] = -0.5 * b2
    for il in range(1, L1):
        cB0 = _cB(il, 0)
        s[:, COL[("q1", il)]] = 3.0 * cB0 * b1
        s[:, COL[("r1", il)]] = cB0 * b1
        s[:, COL[("q2", il)]] = cB0 * b2
    for m in range(1, L1):
        s[:, COL[("b0p", m)]] = m * b0
        s[:, COL[("b0n", m)]] = -m * b0
    for (il, im) in PAIRS:
        cB = _cB(il, im)
        s[:, COL[("c1", il, im)]] = cB * b1
        s[:, COL[("c2p", il, im)]] = cB * b2
        s[:, COL[("c2n", il, im)]] = -cB * b2
    return s


# ---------------------------------------------------------------------------
# The walrus build in this container rejects instructions carrying more than
# ONE sync-wait ("Too many sync wait commands"). Tile's scheduler routinely
# attaches several. Post-pass: hoist all but the last wait of each
# instruction onto same-engine NOPs inserted immediately before it.
def split_multiwaits(nc):
    for f in nc.m.functions:
        for blk in f.blocks:
            new = []
            changed = False
            for ins in blk.instructions:
                si = ins.sync_info
                if si is not None and len(si.on_wait) > 1:
                    waits = list(si.on_wait)
                    for w in waits[:-1]:
                        nop = mybir.InstNoOp(
                            name=nc.get_next_instruction_name(),
                            engine=ins.engine,
                            bass_nofuse=True,
                            sync_info=mybir.SyncInfo(on_wait=[w],
                                                     on_update=[]),
                        )
                        new.append(nop)
                    ins.sync_info = bass_rust.SyncInfo(
                        on_wait=[waits[-1]], on_update=list(si.on_update))
                    changed = True
                new.append(ins)
            if changed:
                blk.instructions = new


# ---------------------------------------------------------------------------
def _slot_products(il, kind, im):
    """Product list [(col_key, (row, slot_kind, slot_idx)), ...] for one
    output slot. rhs refs are symbolic: ('fr'|'fi'|'d0r'|'d0i', k)."""
    if kind == "dr0":
        return [("one", ("d0r", 0)), (("q1", il), ("fr", 1)),
                (("q2", il), ("fi", 1))]
    if kind == "di0":
        return [("one", ("d0i", 0)), (("r1", il), ("fi", 1)),
                (("q2", il), ("fr", 1))]
    if kind == "dr":
        p = [("h1", ("fr", im - 1)), ("h2n", ("fi", im - 1)),
             (("b0p", im), ("fi", im))]
        if im < il:
            p += [(("c1", il, im), ("fr", im + 1)),
                  (("c2n", il, im), ("fi", im + 1))]
        return p
    if kind == "di":
        p = [("h1", ("fi", im - 1)), ("h2p", ("fr", im - 1)),
             (("b0n", im), ("fr", im))]
        if im < il:
            p += [(("c1", il, im), ("fi", im + 1)),
                  (("c2p", il, im), ("fr", im + 1))]
        return p
    raise ValueError(kind)


def build_bass(split=True):
    from bass_rust import add_dep_helper

    nc = bass.Bass()
    pin = nc.dram_tensor("pin", [XS, CIN], BF16, kind="ExternalInput").ap()
    psc = nc.dram_tensor("psc", [XS, NSCAL], F32, kind="ExternalInput").ap()
    pout = nc.dram_tensor("pout", [XS, COUT], BF16, kind="ExternalOutput").ap()

    # slot schedule: row-major, (dr0, di0, dr1, di1, ..)
    slots = []
    for il in range(1, L1):
        slots.append((il, "dr0", 0))
        slots.append((il, "di0", 0))
        for im in range(1, il + 1):
            slots.append((il, "dr", im))
            slots.append((il, "di", im))

    # diag build order: first-use order over the slot schedule
    diag_order = []
    for s in slots:
        for ckey, _ in _slot_products(*s):
            if ckey not in diag_order:
                diag_order.append(ckey)
    assert len(diag_order) == NCOL

    with tile.TileContext(nc) as tc:
        with tc.tile_pool(name="m", bufs=1) as pool, \
             tc.tile_pool(name="p", bufs=1, space="PSUM") as ppool:
            ident = pool.tile([XS, 128], BF16, tag="ident")
            scal = pool.tile([XS, NSCAL], F32, tag="scal")
            diags = pool.tile([XS, NCOL * 128], BF16, tag="diags")

            prev_dma = nc.sync.dma_start(scal[:], psc[:])

            def chain(d):
                nonlocal prev_dma
                add_dep_helper(d.ins, prev_dma.ins,
                               reason="serialize input DMAs")
                prev_dma = d

            chain(nc.sync.dma_start(ident[:], pin[:, 0:128]))

            row_in = {}
            for il in range(1, L1):
                ns = il + 1
                t = pool.tile([XS, (2 * ns + 2) * NV], BF16,
                              name=f"in{il}", tag=f"in{il}")
                o = IN_OFF[il]
                chain(nc.sync.dma_start(t[:], pin[:, o:o + (2 * ns + 2) * NV]))
                row_in[il] = t

            # on-device diagonal weights: diag(col c) = I * scal[:, c]
            dpos = {}
            for i, ckey in enumerate(diag_order):
                dpos[ckey] = i
                nc.vector.tensor_scalar_mul(
                    diags[:, i * 128:(i + 1) * 128], ident[:],
                    scal[:, COL[ckey]:COL[ckey] + 1])

            def dg(ckey):
                i = dpos[ckey]
                return diags[:, i * 128:(i + 1) * 128]

            def rhs_ap(il, ref):
                t = row_in[il]
                ns = il + 1
                kind, k = ref
                base = {"fr": 0, "fi": ns, "d0r": 2 * ns, "d0i": 2 * ns + 1}
                s = base[kind] + k
                return t[:, s * NV:(s + 1) * NV]

            row_out = {}
            for il in range(1, L1):
                ns = il + 1
                row_out[il] = pool.tile([XS, 2 * ns * NV], BF16,
                                        name=f"out{il}", tag=f"out{il}")

            def out_ap(il, kind, im):
                ns = il + 1
                s = im if kind.startswith("dr") else ns + im
                return row_out[il][:, s * NV:(s + 1) * NV]

            banks = [ppool.tile([XS, NV], F32, name=f"bank{i}",
                                 tag=f"bank{i}") for i in range(8)]

            row_last_drain = {}
            for idx, (il, kind, im) in enumerate(slots):
                bank = banks[idx % 8]
                prods = _slot_products(il, kind, im)
                n = len(prods)
                for i, (ckey, ref) in enumerate(prods):
                    nc.tensor.matmul(bank[:], dg(ckey), rhs_ap(il, ref),
                                     start=(i == 0), stop=(i == n - 1))
                dst = out_ap(il, kind, im)
                if idx % 2 == 0:
                    nc.scalar.copy(dst, bank[:])
                else:
                    nc.vector.tensor_copy(dst, bank[:])
                row_last_drain[il] = idx

            for il in range(1, L1):
                ns = il + 1
                nc.sync.dma_start(
                    pout[:, OUT_OFF[il]:OUT_OFF[il] + 2 * ns * NV],
                    row_out[il][:])

    if split:
        split_multiwaits(nc)
    return nc


# ---------------------------------------------------------------------------
def pack_inputs(prev_f_re, prev_f_im, delta0_re, delta0_im, b):
    """-> list of per-core {'pin': [XS, CIN] bf16, 'psc': [XS, NSCAL] f32}."""
    eye = np.eye(XS, dtype=NPBF16)
    in_maps = []
    for c in range(NCORES):
        X = slice(c * XS, (c + 1) * XS)
        p = np.zeros((XS, CIN), NPBF16)
        p[:, 0:128] = eye
        for il in range(1, L1):
            o = IN_OFF[il]
            ns = il + 1
            p[:, o:o + ns * NV] = (
                np.asarray(prev_f_re[il, :ns, X, :], np.float32)
                .transpose(1, 0, 2).reshape(XS, ns * NV).astype(NPBF16))
            o += ns * NV
            p[:, o:o + ns * NV] = (
                np.asarray(prev_f_im[il, :ns, X, :], np.float32)
                .transpose(1, 0, 2).reshape(XS, ns * NV).astype(NPBF16))
            o += ns * NV
            p[:, o:o + NV] = np.asarray(
                delta0_re[il, 0, X, :], np.float32).astype(NPBF16)
            o += NV
            p[:, o:o + NV] = np.asarray(
                delta0_im[il, 0, X, :], np.float32).astype(NPBF16)
        sc = build_scal(np.asarray(b[X], np.float32))
        in_maps.append({"pin": p, "psc": sc})
    return in_maps


def unpack_outputs(results, delta0_re, delta0_im):
    out = np.zeros((L1, L1, NX, NV), np.complex64)
    out[0, 0] = np.asarray(delta0_re[0, 0]) + 1j * np.asarray(delta0_im[0, 0])
    for c in range(NCORES):
        X = slice(c * XS, (c + 1) * XS)
        p = results[c]["pout"].astype(np.float32)
        for il in range(1, L1):
            o = OUT_OFF[il]
            ns = il + 1
            dr = p[:, o:o + ns * NV].reshape(XS, ns, NV).transpose(1, 0, 2)
            di = (p[:, o + ns * NV:o + 2 * ns * NV]
                  .reshape(XS, ns, NV).transpose(1, 0, 2))
            out[il, :ns, X, :] = dr + 1j * di
    return out


_NC_CACHE = None


def get_nc():
    global _NC_CACHE
    if _NC_CACHE is None:
        _NC_CACHE = build_bass()
    return _NC_CACHE


def kernel(prev_f_re, prev_f_im, delta0_re, delta0_im, b, v):
    in_maps = pack_inputs(prev_f_re, prev_f_im, delta0_re, delta0_im, b)
    res = run_bass_kernel_spmd(get_nc(), in_maps, list(range(NCORES)))
    return unpack_outputs(res.results, delta0_re, delta0_im)


# revision 4
# speedup vs baseline: 1.5822x; 1.0999x over previous
"""Trainium2 Bass kernel for nn_Bdfdv_51170240364850 (gnn_message_passing).

Computes, for mode pairs (il, im) with im <= il (L1 = 5 modes each way) and
grid (nx=1024, nv=512):

  D[il,im] = base + (-1j)*im*bx*F[il,im] + cB*bm*F[il,im+1]
             + [im==0] Re(cC*bp*F[il,1])
  base     = 0.5*bm*F[il,im-1]  (il>=1, 1<=im<=il)   else  D0[il,im]

with bx = b[:,0], bm = b[:,1]+1j b[:,2], bp = conj(bm),
cB = -(il-im)(il+im+1)/2, cC = -il(il+1).

Strategy: pure data-parallel over nx across 8 NeuronCores (nx=128 per core on
the 128 SBUF partitions), bf16 end-to-end (tolerance 2e-2; bf16 keeps
~7e-3). Per-x scalar products run on the Tensor engine as diagonal-weight
matmuls accumulating in PSUM:

    out[x, :] += c(x) * in[x, :]   ==   PSUM += diag(c).T @ in

Each of the 28 valid output slots (dr/di per (il, im), row 0 on host) is one
PSUM-bank chain of 2-5 matmuls (N=512). Diagonal [128,128] bf16 weights are
precomputed host-side (products of integer constants with b columns) and
DMA'd; the f32 scalar table also feeds DVE scalar_tensor_tensor drains,
which fold one product (the b0/q2 term) into the PSUM->SBUF drain for rows
3-4, cutting PE work. Rows 1-2 drain as plain ACT copies. Output slots are
pair-interleaved so each (dr,di) pair DMAs out as soon as it drains.
"""

import numpy as np
import ml_dtypes

import bass_rust
import concourse.bass as bass
import concourse.tile as tile
from concourse import mybir
from concourse.bass_utils import run_bass_kernel_spmd

L1 = 5
NX = 1024
NV = 512
NCORES = 8
XS = NX // NCORES  # 128 = SBUF partitions

F32 = mybir.dt.float32
BF16 = mybir.dt.bfloat16
NPBF16 = ml_dtypes.bfloat16

MULT = mybir.AluOpType.mult
ADD = mybir.AluOpType.add

# ---------------------------------------------------------------------------
# coefficient registry
PAIRS = [(2, 1), (3, 1), (3, 2), (4, 1), (4, 2), (4, 3)]  # inner (il, im)


def _cB(il, im):
    return -(il - im) * (il + im + 1) / 2.0


COL = {"one": 0, "h1": 1, "h2p": 2, "h2n": 3}
for _il in range(1, L1):
    COL[("q1", _il)] = 3 + _il           # 4..7
    COL[("r1", _il)] = 7 + _il           # 8..11
    COL[("q2", _il)] = 11 + _il          # 12..15
for _m in range(1, L1):
    COL[("b0p", _m)] = 15 + _m           # 16..19
    COL[("b0n", _m)] = 19 + _m           # 20..23
for _k, _p in enumerate(PAIRS):
    COL[("c1",) + _p] = 24 + 3 * _k
    COL[("c2p",) + _p] = 25 + 3 * _k
    COL[("c2n",) + _p] = 26 + 3 * _k
NSCAL = 48  # padded f32 table width


def coeff_vec(key, b0, b1, b2):
    if key == "one":
        return np.ones_like(b0)
    if key == "h1":
        return 0.5 * b1
    if key == "h2p":
        return 0.5 * b2
    if key == "h2n":
        return -0.5 * b2
    tag = key[0]
    if tag == "q1":
        return 3.0 * _cB(key[1], 0) * b1
    if tag == "r1":
        return _cB(key[1], 0) * b1
    if tag == "q2":
        return _cB(key[1], 0) * b2
    if tag == "b0p":
        return key[1] * b0
    if tag == "b0n":
        return -key[1] * b0
    cB = _cB(key[1], key[2])
    if tag == "c1":
        return cB * b1
    if tag == "c2p":
        return cB * b2
    if tag == "c2n":
        return -cB * b2
    raise ValueError(key)


def build_scal(b_sh):
    """b_sh: [XS, 3] f32 -> [XS, NSCAL] f32 (STT-drain scalars by COL)."""
    b0, b1, b2 = b_sh[:, 0], b_sh[:, 1], b_sh[:, 2]
    s = np.zeros((XS, NSCAL), np.float32)
    for key, c in COL.items():
        s[:, c] = coeff_vec(key, b0, b1, b2)
    return s


# ---------------------------------------------------------------------------
# slot schedule and product table
def _slot_products(il, kind, im):
    """[(col_key, ('fr'|'fi'|'d0r'|'d0i', k)), ...] full product list."""
    if kind == "dr0":
        return [("one", ("d0r", 0)), (("q1", il), ("fr", 1)),
                (("q2", il), ("fi", 1))]
    if kind == "di0":
        return [("one", ("d0i", 0)), (("r1", il), ("fi", 1)),
                (("q2", il), ("fr", 1))]
    if kind == "dr":
        p = [("h1", ("fr", im - 1)), ("h2n", ("fi", im - 1)),
             (("b0p", im), ("fi", im))]
        if im < il:
            p += [(("c1", il, im), ("fr", im + 1)),
                  (("c2n", il, im), ("fi", im + 1))]
        return p
    if kind == "di":
        p = [("h1", ("fi", im - 1)), ("h2p", ("fr", im - 1)),
             (("b0n", im), ("fr", im))]
        if im < il:
            p += [(("c1", il, im), ("fi", im + 1)),
                  (("c2p", il, im), ("fr", im + 1))]
        return p
    raise ValueError(kind)


SLOTS = []
for _il in range(1, L1):
    SLOTS.append((_il, "dr0", 0))
    SLOTS.append((_il, "di0", 0))
    for _im in range(1, _il + 1):
        SLOTS.append((_il, "dr", _im))
        SLOTS.append((_il, "di", _im))

# STT-drain (DVE, folds one product into the drain) for rows 3-4; ACT copy
# drains for rows 1-2. The folded product is the last in _slot_products
# (b0 term) for im>=1, the q2 term for im=0.


def _drain_plan(il, kind, im):
    """-> (pe_products, stt_fold or None). stt_fold = (col_key, rhs_ref)."""
    prods = _slot_products(il, kind, im)
    if il >= 3:
        if kind in ("dr0", "di0"):
            return prods[:2], prods[2]
        # drop the b0 product (index 2)
        return prods[:2] + prods[3:], prods[2]
    return prods, None


# diag keys actually used on PE, in first-use order
DIAG_ORDER = []
for _s in SLOTS:
    for _ck, _ in _drain_plan(*_s)[0]:
        if _ck not in DIAG_ORDER:
            DIAG_ORDER.append(_ck)
NDIAG = len(DIAG_ORDER)
DIAG_POS = {k: i for i, k in enumerate(DIAG_ORDER)}
# diags needed by row 1's slots (DMA'd first, before row-1 data)
_row1_keys = []
for _s in SLOTS[:4]:
    for _ck, _ in _drain_plan(*_s)[0]:
        if _ck not in _row1_keys:
            _row1_keys.append(_ck)
NDIAG_A = len(_row1_keys)
assert _row1_keys == DIAG_ORDER[:NDIAG_A]

# packed input layout (bf16):
# [diagsA (row1's, NDIAG_A*128) | row1 block | diagsB (rest) | rows 2..4]
# row block: fr slots (ns), fi slots (ns), d0r, d0i  -- each slot NV cols
DIAGS_A_OFF = 0
ROW1_OFF = NDIAG_A * 128
DIAGS_B_OFF = ROW1_OFF + (2 * 2 + 2) * NV
IN_OFF = {1: ROW1_OFF}
_o = DIAGS_B_OFF + (NDIAG - NDIAG_A) * 128
for _il in range(2, L1):
    IN_OFF[_il] = _o
    _o += (2 * (_il + 1) + 2) * NV
CIN = _o

# packed output layout (bf16): row blocks il=1..4, pair-interleaved:
# [dr0, di0, dr1, di1, ...]
OUT_OFF = {}
_o = 0
for _il in range(1, L1):
    OUT_OFF[_il] = _o
    _o += 2 * (_il + 1) * NV
COUT = _o


# ---------------------------------------------------------------------------
# walrus in this container rejects >1 sync-wait per instruction; hoist
# extras onto same-engine NOPs.
def split_multiwaits(nc):
    for f in nc.m.functions:
        for blk in f.blocks:
            new = []
            changed = False
            for ins in blk.instructions:
                si = ins.sync_info
                if si is not None and len(si.on_wait) > 1:
                    waits = list(si.on_wait)
                    for w in waits[:-1]:
                        nop = mybir.InstNoOp(
                            name=nc.get_next_instruction_name(),
                            engine=ins.engine,
                            bass_nofuse=True,
                            sync_info=mybir.SyncInfo(on_wait=[w],
                                                     on_update=[]),
                        )
                        new.append(nop)
                    ins.sync_info = bass_rust.SyncInfo(
                        on_wait=[waits[-1]], on_update=list(si.on_update))
                    changed = True
                new.append(ins)
            if changed:
                blk.instructions = new


# ---------------------------------------------------------------------------
def build_bass(split=True):
    from bass_rust import add_dep_helper

    nc = bass.Bass()
    pin = nc.dram_tensor("pin", [XS, CIN], BF16, kind="ExternalInput").ap()
    psc = nc.dram_tensor("psc", [XS, NSCAL], F32, kind="ExternalInput").ap()
    pout = nc.dram_tensor("pout", [XS, COUT], BF16, kind="ExternalOutput").ap()

    with tile.TileContext(nc) as tc:
        with tc.tile_pool(name="m", bufs=1) as pool, \
             tc.tile_pool(name="p", bufs=1, space="PSUM") as ppool:
            scal = pool.tile([XS, NSCAL], F32, tag="scal")
            diags = pool.tile([XS, NDIAG * 128], BF16, tag="diags")

            prev_dma = nc.sync.dma_start(scal[:], psc[:])

            def chain(d):
                nonlocal prev_dma
                add_dep_helper(d.ins, prev_dma.ins,
                               reason="serialize input DMAs")
                prev_dma = d

            chain(nc.sync.dma_start(diags[:, 0:NDIAG_A * 128],
                                    pin[:, 0:NDIAG_A * 128]))

            row_in = {}
            t1 = pool.tile([XS, (2 * 2 + 2) * NV], BF16, name="in1",
                           tag="in1")
            chain(nc.sync.dma_start(
                t1[:], pin[:, ROW1_OFF:ROW1_OFF + (2 * 2 + 2) * NV]))
            row_in[1] = t1
            chain(nc.sync.dma_start(
                diags[:, NDIAG_A * 128:NDIAG * 128],
                pin[:, DIAGS_B_OFF:DIAGS_B_OFF + (NDIAG - NDIAG_A) * 128]))
            for il in range(2, L1):
                ns = il + 1
                t = pool.tile([XS, (2 * ns + 2) * NV], BF16,
                              name=f"in{il}", tag=f"in{il}")
                o = IN_OFF[il]
                chain(nc.sync.dma_start(t[:], pin[:, o:o + (2 * ns + 2) * NV]))
                row_in[il] = t

            def dg(ckey):
                i = DIAG_POS[ckey]
                return diags[:, i * 128:(i + 1) * 128]

            def rhs_ap(il, ref):
                t = row_in[il]
                ns = il + 1
                kind, k = ref
                base = {"fr": 0, "fi": ns, "d0r": 2 * ns, "d0i": 2 * ns + 1}
                s = base[kind] + k
                return t[:, s * NV:(s + 1) * NV]

            row_out = {}
            for il in range(1, L1):
                ns = il + 1
                row_out[il] = pool.tile([XS, 2 * ns * NV], BF16,
                                        name=f"out{il}", tag=f"out{il}")

            def out_ap(il, kind, im):
                # pair-interleaved: slot index = 2*im + (0 if dr else 1)
                s = 2 * im + (0 if kind.startswith("dr") else 1)
                return row_out[il][:, s * NV:(s + 1) * NV]

            banks = [ppool.tile([XS, NV], F32, name=f"bank{i}",
                                tag=f"bank{i}") for i in range(8)]

            for idx, (il, kind, im) in enumerate(SLOTS):
                bank = banks[idx % 8]
                pe_prods, fold = _drain_plan(il, kind, im)
                n = len(pe_prods)
                for i, (ckey, ref) in enumerate(pe_prods):
                    nc.tensor.matmul(bank[:], dg(ckey), rhs_ap(il, ref),
                                     start=(i == 0), stop=(i == n - 1))
                dst = out_ap(il, kind, im)
                if fold is not None:
                    ckey, ref = fold
                    c = COL[ckey]
                    nc.vector.scalar_tensor_tensor(
                        dst, rhs_ap(il, ref), scal[:, c:c + 1], bank[:],
                        MULT, ADD)
                else:
                    nc.scalar.copy(dst, bank[:])
                # pair-complete -> DMA the (dr, di) pair out
                if kind in ("di0", "di"):
                    ns = il + 1
                    o = OUT_OFF[il] + 2 * im * NV
                    nc.sync.dma_start(
                        pout[:, o:o + 2 * NV],
                        row_out[il][:, 2 * im * NV:(2 * im + 2) * NV])

    if split:
        split_multiwaits(nc)
    return nc


# ---------------------------------------------------------------------------
def pack_inputs(prev_f_re, prev_f_im, delta0_re, delta0_im, b):
    """-> per-core {'pin': [XS, CIN] bf16, 'psc': [XS, NSCAL] f32}."""
    in_maps = []
    for c in range(NCORES):
        X = slice(c * XS, (c + 1) * XS)
        b_sh = np.asarray(b[X], np.float32)
        b0, b1, b2 = b_sh[:, 0], b_sh[:, 1], b_sh[:, 2]
        p = np.zeros((XS, CIN), NPBF16)
        for i, key in enumerate(DIAG_ORDER):
            off = (DIAGS_A_OFF + i * 128 if i < NDIAG_A
                   else DIAGS_B_OFF + (i - NDIAG_A) * 128)
            d = p[:, off:off + 128]
            np.fill_diagonal(d, coeff_vec(key, b0, b1, b2).astype(NPBF16))
        for il in range(1, L1):
            o = IN_OFF[il]
            ns = il + 1
            p[:, o:o + ns * NV] = (
                np.asarray(prev_f_re[il, :ns, X, :], np.float32)
                .transpose(1, 0, 2).reshape(XS, ns * NV).astype(NPBF16))
            o += ns * NV
            p[:, o:o + ns * NV] = (
                np.asarray(prev_f_im[il, :ns, X, :], np.float32)
                .transpose(1, 0, 2).reshape(XS, ns * NV).astype(NPBF16))
            o += ns * NV
            p[:, o:o + NV] = np.asarray(
                delta0_re[il, 0, X, :], np.float32).astype(NPBF16)
            o += NV
            p[:, o:o + NV] = np.asarray(
                delta0_im[il, 0, X, :], np.float32).astype(NPBF16)
        in_maps.append({"pin": p, "psc": build_scal(b_sh)})
    return in_maps


def unpack_outputs(results, delta0_re, delta0_im):
    out = np.zeros((L1, L1, NX, NV), np.complex64)
    out[0, 0] = np.asarray(delta0_re[0, 0]) + 1j * np.asarray(delta0_im[0, 0])
    for c in range(NCORES):
        X = slice(c * XS, (c + 1) * XS)
        p = results[c]["pout"].astype(np.float32)
        for il in range(1, L1):
            o = OUT_OFF[il]
            ns = il + 1
            blk = p[:, o:o + 2 * ns * NV].reshape(XS, ns, 2, NV)
            dr = blk[:, :, 0, :].transpose(1, 0, 2)
            di = blk[:, :, 1, :].transpose(1, 0, 2)
            out[il, :ns, X, :] = dr + 1j * di
    return out


_NC_CACHE = None


def get_nc():
    global _NC_CACHE
    if _NC_CACHE is None:
        _NC_CACHE = build_bass()
    return _NC_CACHE


def kernel(prev_f_re, prev_f_im, delta0_re, delta0_im, b, v):
    in_maps = pack_inputs(prev_f_re, prev_f_im, delta0_re, delta0_im, b)
    res = run_bass_kernel_spmd(get_nc(), in_maps, list(range(NCORES)))
    return unpack_outputs(res.results, delta0_re, delta0_im)


# revision 5
# speedup vs baseline: 1.7456x; 1.1033x over previous
"""Trainium2 Bass kernel for nn_Bdfdv_51170240364850 (gnn_message_passing).

Computes, for mode pairs (il, im) with im <= il (L1 = 5 modes each way) and
grid (nx=1024, nv=512):

  D[il,im] = base + (-1j)*im*bx*F[il,im] + cB*bm*F[il,im+1]
             + [im==0] Re(cC*bp*F[il,1])
  base     = 0.5*bm*F[il,im-1]  (il>=1, 1<=im<=il)   else  D0[il,im]

with bx = b[:,0], bm = b[:,1]+1j b[:,2], bp = conj(bm),
cB = -(il-im)(il+im+1)/2, cC = -il(il+1).

Strategy: pure data-parallel over nx across 8 NeuronCores (nx=128 per core on
the 128 SBUF partitions), bf16 end-to-end (tolerance 2e-2; bf16 keeps
~7e-3). Per-x scalar products run on the Tensor engine as diagonal-weight
matmuls accumulating in PSUM:

    out[x, :] += c(x) * in[x, :]   ==   PSUM += diag(c).T @ in

Each of the 28 valid output slots (dr/di per (il, im), row 0 on host) is one
PSUM-bank chain of 2-5 matmuls (N=512). Diagonal [128,128] bf16 weights are
precomputed host-side (products of integer constants with b columns) and
DMA'd; the f32 scalar table also feeds DVE scalar_tensor_tensor drains,
which fold one product (the b0/q2 term) into the PSUM->SBUF drain for rows
3-4, cutting PE work. Rows 1-2 drain as plain ACT copies. Output slots are
pair-interleaved so each (dr,di) pair DMAs out as soon as it drains.
"""

import numpy as np
import ml_dtypes

import bass_rust
import concourse.bass as bass
import concourse.tile as tile
from concourse import mybir
from concourse.bass_utils import run_bass_kernel_spmd

L1 = 5
NX = 1024
NV = 512
NCORES = 8
XS = NX // NCORES  # 128 = SBUF partitions

F32 = mybir.dt.float32
BF16 = mybir.dt.bfloat16
NPBF16 = ml_dtypes.bfloat16

MULT = mybir.AluOpType.mult
ADD = mybir.AluOpType.add

# ---------------------------------------------------------------------------
# coefficient registry
PAIRS = [(2, 1), (3, 1), (3, 2), (4, 1), (4, 2), (4, 3)]  # inner (il, im)


def _cB(il, im):
    return -(il - im) * (il + im + 1) / 2.0


COL = {"one": 0, "h1": 1, "h2p": 2, "h2n": 3}
for _il in range(1, L1):
    COL[("q1", _il)] = 3 + _il           # 4..7
    COL[("r1", _il)] = 7 + _il           # 8..11
    COL[("q2", _il)] = 11 + _il          # 12..15
for _m in range(1, L1):
    COL[("b0p", _m)] = 15 + _m           # 16..19
    COL[("b0n", _m)] = 19 + _m           # 20..23
for _k, _p in enumerate(PAIRS):
    COL[("c1",) + _p] = 24 + 3 * _k
    COL[("c2p",) + _p] = 25 + 3 * _k
    COL[("c2n",) + _p] = 26 + 3 * _k
NSCAL = 48  # padded f32 table width


def coeff_vec(key, b0, b1, b2):
    if key == "one":
        return np.ones_like(b0)
    if key == "h1":
        return 0.5 * b1
    if key == "h2p":
        return 0.5 * b2
    if key == "h2n":
        return -0.5 * b2
    tag = key[0]
    if tag == "q1":
        return 3.0 * _cB(key[1], 0) * b1
    if tag == "r1":
        return _cB(key[1], 0) * b1
    if tag == "q2":
        return _cB(key[1], 0) * b2
    if tag == "b0p":
        return key[1] * b0
    if tag == "b0n":
        return -key[1] * b0
    cB = _cB(key[1], key[2])
    if tag == "c1":
        return cB * b1
    if tag == "c2p":
        return cB * b2
    if tag == "c2n":
        return -cB * b2
    raise ValueError(key)


def build_scal(b_sh):
    """b_sh: [XS, 3] f32 -> [XS, NSCAL] f32 (STT-drain scalars by COL)."""
    b0, b1, b2 = b_sh[:, 0], b_sh[:, 1], b_sh[:, 2]
    s = np.zeros((XS, NSCAL), np.float32)
    for key, c in COL.items():
        s[:, c] = coeff_vec(key, b0, b1, b2)
    return s


# ---------------------------------------------------------------------------
# slot schedule and product table
def _slot_products(il, kind, im):
    """[(col_key, ('fr'|'fi'|'d0r'|'d0i', k)), ...] full product list."""
    if kind == "dr0":
        return [("one", ("d0r", 0)), (("q1", il), ("fr", 1)),
                (("q2", il), ("fi", 1))]
    if kind == "di0":
        return [("one", ("d0i", 0)), (("r1", il), ("fi", 1)),
                (("q2", il), ("fr", 1))]
    if kind == "dr":
        p = [("h1", ("fr", im - 1)), ("h2n", ("fi", im - 1)),
             (("b0p", im), ("fi", im))]
        if im < il:
            p += [(("c1", il, im), ("fr", im + 1)),
                  (("c2n", il, im), ("fi", im + 1))]
        return p
    if kind == "di":
        p = [("h1", ("fi", im - 1)), ("h2p", ("fr", im - 1)),
             (("b0n", im), ("fr", im))]
        if im < il:
            p += [(("c1", il, im), ("fi", im + 1)),
                  (("c2p", il, im), ("fr", im + 1))]
        return p
    raise ValueError(kind)


SLOTS = []
for _il in range(1, L1):
    SLOTS.append((_il, "dr0", 0))
    SLOTS.append((_il, "di0", 0))
    for _im in range(1, _il + 1):
        SLOTS.append((_il, "dr", _im))
        SLOTS.append((_il, "di", _im))

# STT-drain (DVE, folds one product into the drain) for rows 3-4; ACT copy
# drains for rows 1-2. The folded product is the last in _slot_products
# (b0 term) for im>=1, the q2 term for im=0.


def _drain_plan(il, kind, im):
    """-> (pe_products, stt_fold or None). stt_fold = (col_key, rhs_ref)."""
    prods = _slot_products(il, kind, im)
    if il >= 3:
        if kind in ("dr0", "di0"):
            return prods[:2], prods[2]
        # drop the b0 product (index 2)
        return prods[:2] + prods[3:], prods[2]
    return prods, None


# diag keys actually used on PE, in first-use order
DIAG_ORDER = []
for _s in SLOTS:
    for _ck, _ in _drain_plan(*_s)[0]:
        if _ck not in DIAG_ORDER:
            DIAG_ORDER.append(_ck)
NDIAG = len(DIAG_ORDER)
DIAG_POS = {k: i for i, k in enumerate(DIAG_ORDER)}
# diags needed by row 1's slots (DMA'd first, before row-1 data)
_row1_keys = []
for _s in SLOTS[:4]:
    for _ck, _ in _drain_plan(*_s)[0]:
        if _ck not in _row1_keys:
            _row1_keys.append(_ck)
NDIAG_A = len(_row1_keys)
assert _row1_keys == DIAG_ORDER[:NDIAG_A]

# packed input layout (bf16): [identity (128 cols) | row blocks il=1..4]
# row block: fr slots (ns), fi slots (ns), d0r, d0i  -- each slot NV cols
IN_OFF = {}
_o = 128
for _il in range(1, L1):
    IN_OFF[_il] = _o
    _o += (2 * (_il + 1) + 2) * NV
CIN = _o

# packed output layout (bf16): row blocks il=1..4, pair-interleaved:
# [dr0, di0, dr1, di1, ...]
OUT_OFF = {}
_o = 0
for _il in range(1, L1):
    OUT_OFF[_il] = _o
    _o += 2 * (_il + 1) * NV
COUT = _o


# ---------------------------------------------------------------------------
# walrus in this container rejects >1 sync-wait per instruction; hoist
# extras onto same-engine NOPs.
def split_multiwaits(nc):
    for f in nc.m.functions:
        for blk in f.blocks:
            new = []
            changed = False
            for ins in blk.instructions:
                si = ins.sync_info
                if si is not None and len(si.on_wait) > 1:
                    waits = list(si.on_wait)
                    for w in waits[:-1]:
                        nop = mybir.InstNoOp(
                            name=nc.get_next_instruction_name(),
                            engine=ins.engine,
                            bass_nofuse=True,
                            sync_info=mybir.SyncInfo(on_wait=[w],
                                                     on_update=[]),
                        )
                        new.append(nop)
                    ins.sync_info = bass_rust.SyncInfo(
                        on_wait=[waits[-1]], on_update=list(si.on_update))
                    changed = True
                new.append(ins)
            if changed:
                blk.instructions = new


# ---------------------------------------------------------------------------
def build_bass(split=True):
    from bass_rust import add_dep_helper

    nc = bass.Bass()
    pin = nc.dram_tensor("pin", [XS, CIN], BF16, kind="ExternalInput").ap()
    psc = nc.dram_tensor("psc", [XS, NSCAL], F32, kind="ExternalInput").ap()
    pout = nc.dram_tensor("pout", [XS, COUT], BF16, kind="ExternalOutput").ap()

    with tile.TileContext(nc) as tc:
        with tc.tile_pool(name="m", bufs=1) as pool, \
             tc.tile_pool(name="p", bufs=1, space="PSUM") as ppool:
            scal = pool.tile([XS, NSCAL], F32, tag="scal")
            ident = pool.tile([XS, 128], BF16, tag="ident")
            diags = pool.tile([XS, NDIAG * 128], BF16, tag="diags")

            # issue order doubles as queue priority: scal, ident, rows 1..4
            nc.sync.dma_start(scal[:], psc[:])
            nc.sync.dma_start(ident[:], pin[:, 0:128])

            row_in = {}
            for il in range(1, L1):
                ns = il + 1
                t = pool.tile([XS, (2 * ns + 2) * NV], BF16,
                              name=f"in{il}", tag=f"in{il}")
                o = IN_OFF[il]
                nc.sync.dma_start(t[:], pin[:, o:o + (2 * ns + 2) * NV])
                row_in[il] = t

            # on-device diagonal weights: diag(col c) = ident * scal[:, c]
            for i, ckey in enumerate(DIAG_ORDER):
                nc.vector.tensor_scalar_mul(
                    diags[:, i * 128:(i + 1) * 128], ident[:],
                    scal[:, COL[ckey]:COL[ckey] + 1])

            # PE warm-up: ramp the pstate before real work arrives
            wbank = ppool.tile([XS, NV], F32, name="wbank", tag="wbank")
            for _ in range(14):
                nc.tensor.matmul(wbank[:], ident[:], row_in[1][:, 0:NV],
                                 start=True, stop=True)

            def dg(ckey):
                i = DIAG_POS[ckey]
                return diags[:, i * 128:(i + 1) * 128]

            def rhs_ap(il, ref):
                t = row_in[il]
                ns = il + 1
                kind, k = ref
                base = {"fr": 0, "fi": ns, "d0r": 2 * ns, "d0i": 2 * ns + 1}
                s = base[kind] + k
                return t[:, s * NV:(s + 1) * NV]

            row_out = {}
            for il in range(1, L1):
                ns = il + 1
                row_out[il] = pool.tile([XS, 2 * ns * NV], BF16,
                                        name=f"out{il}", tag=f"out{il}")

            def out_ap(il, kind, im):
                # pair-interleaved: slot index = 2*im + (0 if dr else 1)
                s = 2 * im + (0 if kind.startswith("dr") else 1)
                return row_out[il][:, s * NV:(s + 1) * NV]

            banks = [ppool.tile([XS, NV], F32, name=f"bank{i}",
                                tag=f"bank{i}") for i in range(7)]

            for idx, (il, kind, im) in enumerate(SLOTS):
                bank = banks[idx % 7]
                pe_prods, fold = _drain_plan(il, kind, im)
                n = len(pe_prods)
                for i, (ckey, ref) in enumerate(pe_prods):
                    nc.tensor.matmul(bank[:], dg(ckey), rhs_ap(il, ref),
                                     start=(i == 0), stop=(i == n - 1))
                dst = out_ap(il, kind, im)
                if fold is not None:
                    ckey, ref = fold
                    c = COL[ckey]
                    nc.vector.scalar_tensor_tensor(
                        dst, rhs_ap(il, ref), scal[:, c:c + 1], bank[:],
                        MULT, ADD)
                else:
                    nc.scalar.copy(dst, bank[:])
                # pair-complete -> DMA the (dr, di) pair out
                if kind in ("di0", "di"):
                    ns = il + 1
                    o = OUT_OFF[il] + 2 * im * NV
                    nc.sync.dma_start(
                        pout[:, o:o + 2 * NV],
                        row_out[il][:, 2 * im * NV:(2 * im + 2) * NV])

    if split:
        split_multiwaits(nc)
    return nc


# ---------------------------------------------------------------------------
def pack_inputs(prev_f_re, prev_f_im, delta0_re, delta0_im, b):
    """-> per-core {'pin': [XS, CIN] bf16, 'psc': [XS, NSCAL] f32}."""
    in_maps = []
    for c in range(NCORES):
        X = slice(c * XS, (c + 1) * XS)
        b_sh = np.asarray(b[X], np.float32)
        p = np.zeros((XS, CIN), NPBF16)
        p[:, 0:128] = np.eye(XS, dtype=NPBF16)
        for il in range(1, L1):
            o = IN_OFF[il]
            ns = il + 1
            p[:, o:o + ns * NV] = (
                np.asarray(prev_f_re[il, :ns, X, :], np.float32)
                .transpose(1, 0, 2).reshape(XS, ns * NV).astype(NPBF16))
            o += ns * NV
            p[:, o:o + ns * NV] = (
                np.asarray(prev_f_im[il, :ns, X, :], np.float32)
                .transpose(1, 0, 2).reshape(XS, ns * NV).astype(NPBF16))
            o += ns * NV
            p[:, o:o + NV] = np.asarray(
                delta0_re[il, 0, X, :], np.float32).astype(NPBF16)
            o += NV
            p[:, o:o + NV] = np.asarray(
                delta0_im[il, 0, X, :], np.float32).astype(NPBF16)
        in_maps.append({"pin": p, "psc": build_scal(b_sh)})
    return in_maps


def unpack_outputs(results, delta0_re, delta0_im):
    out = np.zeros((L1, L1, NX, NV), np.complex64)
    out[0, 0] = np.asarray(delta0_re[0, 0]) + 1j * np.asarray(delta0_im[0, 0])
    for c in range(NCORES):
        X = slice(c * XS, (c + 1) * XS)
        p = results[c]["pout"].astype(np.float32)
        for il in range(1, L1):
            o = OUT_OFF[il]
            ns = il + 1
            blk = p[:, o:o + 2 * ns * NV].reshape(XS, ns, 2, NV)
            dr = blk[:, :, 0, :].transpose(1, 0, 2)
            di = blk[:, :, 1, :].transpose(1, 0, 2)
            out[il, :ns, X, :] = dr + 1j * di
    return out


_NC_CACHE = None


def get_nc():
    global _NC_CACHE
    if _NC_CACHE is None:
        _NC_CACHE = build_bass()
    return _NC_CACHE


def kernel(prev_f_re, prev_f_im, delta0_re, delta0_im, b, v):
    in_maps = pack_inputs(prev_f_re, prev_f_im, delta0_re, delta0_im, b)
    res = run_bass_kernel_spmd(get_nc(), in_maps, list(range(NCORES)))
    return unpack_outputs(res.results, delta0_re, delta0_im)


# revision 6
# speedup vs baseline: 2.0247x; 1.1599x over previous
"""Trainium2 Bass kernel for nn_Bdfdv_51170240364850 (gnn_message_passing).

Computes, for mode pairs (il, im) with im <= il (L1 = 5 modes each way) and
grid (nx=1024, nv=512):

  D[il,im] = base + (-1j)*im*bx*F[il,im] + cB*bm*F[il,im+1]
             + [im==0] Re(cC*bp*F[il,1])
  base     = 0.5*bm*F[il,im-1]  (il>=1, 1<=im<=il)   else  D0[il,im]

with bx = b[:,0], bm = b[:,1]+1j b[:,2], bp = conj(bm),
cB = -(il-im)(il+im+1)/2, cC = -il(il+1).

Strategy: pure data-parallel over nx across 8 NeuronCores (nx=128 per core on
the 128 SBUF partitions), bf16 end-to-end (tolerance 2e-2; bf16 keeps
~7e-3). Per-x scalar products run on the Tensor engine as diagonal-weight
matmuls accumulating in PSUM:

    out[x, :] += c(x) * in[x, :]   ==   PSUM += diag(c).T @ in

Each of the 28 valid output slots (dr/di per (il, im), row 0 on host) is one
PSUM-bank chain of 2-5 matmuls (N=512). Work split that keeps every engine
busy:

 - PE: diagonal matmuls (74 + warm-up), diagonals built on-device from an
   identity tile x f32 coefficient columns (DVE/ACT tensor_scalar, ~200ns).
 - DVE: diag builds, G-feed tensor_scalar ops, and scalar_tensor_tensor
   drains for rows 3-4 that FOLD one product (b0/q2 term) into the
   PSUM->SBUF drain, cutting PE work.
 - ACT: remaining diag builds + plain copy drains (rows 1-2, row 3 im=0 and
   edge).
 - Pool (GpSimd): builds G = F[im-1] + 2cB*F[im+1] (constant integer
   coefficients) for rows 3-4 inner slots via tensor_tensor adds, so each
   inner slot needs only 2 matmuls (h1*G, -/+h2*G~) instead of 4.

Matmuls are emitted in waves of 7 PSUM banks, grouped by diagonal within a
wave to maximize stationary-weight reuse. Output slots are pair-interleaved
and DMA out per (dr,di) pair as soon as drained.
"""

import numpy as np
import ml_dtypes

import bass_rust
import concourse.bass as bass
import concourse.tile as tile
from concourse import mybir
from concourse.bass_utils import run_bass_kernel_spmd

L1 = 5
NX = 1024
NV = 512
NCORES = 8
XS = NX // NCORES  # 128 = SBUF partitions

F32 = mybir.dt.float32
BF16 = mybir.dt.bfloat16
NPBF16 = ml_dtypes.bfloat16

MULT = mybir.AluOpType.mult
ADD = mybir.AluOpType.add

# ---------------------------------------------------------------------------
# coefficient registry
PAIRS = [(2, 1), (3, 1), (3, 2), (4, 1), (4, 2), (4, 3)]  # inner (il, im)


def _cB(il, im):
    return -(il - im) * (il + im + 1) / 2.0


COL = {"one": 0, "h1": 1, "h2p": 2, "h2n": 3}
for _il in range(1, L1):
    COL[("q1", _il)] = 3 + _il           # 4..7
    COL[("r1", _il)] = 7 + _il           # 8..11
    COL[("q2", _il)] = 11 + _il          # 12..15
for _m in range(1, L1):
    COL[("b0p", _m)] = 15 + _m           # 16..19
    COL[("b0n", _m)] = 19 + _m           # 20..23
for _k, _p in enumerate(PAIRS):
    COL[("c1",) + _p] = 24 + 3 * _k
    COL[("c2p",) + _p] = 25 + 3 * _k
    COL[("c2n",) + _p] = 26 + 3 * _k
NSCAL = 48  # padded f32 table width


def coeff_vec(key, b0, b1, b2):
    if key == "one":
        return np.ones_like(b0)
    if key == "h1":
        return 0.5 * b1
    if key == "h2p":
        return 0.5 * b2
    if key == "h2n":
        return -0.5 * b2
    tag = key[0]
    if tag == "q1":
        return 3.0 * _cB(key[1], 0) * b1
    if tag == "r1":
        return _cB(key[1], 0) * b1
    if tag == "q2":
        return _cB(key[1], 0) * b2
    if tag == "b0p":
        return key[1] * b0
    if tag == "b0n":
        return -key[1] * b0
    cB = _cB(key[1], key[2])
    if tag == "c1":
        return cB * b1
    if tag == "c2p":
        return cB * b2
    if tag == "c2n":
        return -cB * b2
    raise ValueError(key)


def build_scal(b_sh):
    """b_sh: [XS, 3] f32 -> [XS, NSCAL] f32 (STT-drain scalars by COL)."""
    b0, b1, b2 = b_sh[:, 0], b_sh[:, 1], b_sh[:, 2]
    s = np.zeros((XS, NSCAL), np.float32)
    for key, c in COL.items():
        s[:, c] = coeff_vec(key, b0, b1, b2)
    return s


# ---------------------------------------------------------------------------
# slot schedule: which products run on PE, which fold into the DVE drain,
# and which inner slots read the Pool-built G tiles.
SLOTS = []
for _il in range(1, L1):
    SLOTS.append((_il, "dr0", 0))
    SLOTS.append((_il, "di0", 0))
    for _im in range(1, _il + 1):
        SLOTS.append((_il, "dr", _im))
        SLOTS.append((_il, "di", _im))

G_ROWS = (3, 4)          # rows whose inner slots use G tiles
G_PAIRS = [(il, im) for il in G_ROWS for im in range(1, il)]


def _is_kill(il, kind, im):
    """STT-drain (DVE) slots: fold the b0/q2 product into the drain."""
    return il == 4 or (il == 3 and kind in ("dr", "di") and im < il)


def _pe_plan(il, kind, im):
    """-> (pe_products, fold or None); products are (col_key, rhs_ref),
    rhs_ref = (space, k) with space in fr/fi/gr/gi/d0r/d0i."""
    kill = _is_kill(il, kind, im)
    if kind == "dr0":
        pe = [("one", ("d0r", 0)), (("q1", il), ("fr", 1))]
        last = (("q2", il), ("fi", 1))
    elif kind == "di0":
        pe = [("one", ("d0i", 0)), (("r1", il), ("fi", 1))]
        last = (("q2", il), ("fr", 1))
    elif kind == "dr":
        if im < il and il in G_ROWS:
            pe = [("h1", ("gr", im)), ("h2n", ("gi", im))]
        elif im < il:
            pe = [("h1", ("fr", im - 1)), ("h2n", ("fi", im - 1)),
                  (("c1", il, im), ("fr", im + 1)),
                  (("c2n", il, im), ("fi", im + 1))]
        else:
            pe = [("h1", ("fr", im - 1)), ("h2n", ("fi", im - 1))]
        last = ((("b0p", im)), ("fi", im))
    elif kind == "di":
        if im < il and il in G_ROWS:
            pe = [("h1", ("gi", im)), ("h2p", ("gr", im))]
        elif im < il:
            pe = [("h1", ("fi", im - 1)), ("h2p", ("fr", im - 1)),
                  (("c1", il, im), ("fi", im + 1)),
                  (("c2p", il, im), ("fr", im + 1))]
        else:
            pe = [("h1", ("fi", im - 1)), ("h2p", ("fr", im - 1))]
        last = ((("b0n", im)), ("fr", im))
    else:
        raise ValueError(kind)
    if kill:
        return pe, last
    return pe + [last], None


# diag keys used on PE, in first-use order
DIAG_ORDER = []
for _s in SLOTS:
    for _ck, _ in _pe_plan(*_s)[0]:
        if _ck not in DIAG_ORDER:
            DIAG_ORDER.append(_ck)
NDIAG = len(DIAG_ORDER)
DIAG_POS = {k: i for i, k in enumerate(DIAG_ORDER)}
NDIAG_DVE = 12           # first N built on DVE, rest on ACT

# packed input layout (bf16): [identity (128 cols) | row blocks il=1..4]
# row block: fr slots (ns), fi slots (ns), d0r, d0i  -- each slot NV cols
IN_OFF = {}
_o = 128
for _il in range(1, L1):
    IN_OFF[_il] = _o
    _o += (2 * (_il + 1) + 2) * NV
CIN = _o

# packed output layout (bf16): row blocks il=1..4, pair-interleaved:
# [dr0, di0, dr1, di1, ...]
OUT_OFF = {}
_o = 0
for _il in range(1, L1):
    OUT_OFF[_il] = _o
    _o += 2 * (_il + 1) * NV
COUT = _o


# ---------------------------------------------------------------------------
# walrus in this container rejects >1 sync-wait per instruction; hoist
# extras onto same-engine NOPs.
def split_multiwaits(nc):
    for f in nc.m.functions:
        for blk in f.blocks:
            new = []
            changed = False
            for ins in blk.instructions:
                si = ins.sync_info
                if si is not None and len(si.on_wait) > 1:
                    waits = list(si.on_wait)
                    for w in waits[:-1]:
                        nop = mybir.InstNoOp(
                            name=nc.get_next_instruction_name(),
                            engine=ins.engine,
                            bass_nofuse=True,
                            sync_info=mybir.SyncInfo(on_wait=[w],
                                                     on_update=[]),
                        )
                        new.append(nop)
                    ins.sync_info = bass_rust.SyncInfo(
                        on_wait=[waits[-1]], on_update=list(si.on_update))
                    changed = True
                new.append(ins)
            if changed:
                blk.instructions = new


def _pair(ap, step_elems, nblocks=2):
    """Contiguous [P, L] AP -> [P, nblocks, L] with element step between
    blocks."""
    c = ap.copy()
    v = c.ap
    last = v.pop()
    v.append((step_elems, nblocks))
    v.append(tuple(last))
    c.ap = v
    return c


# ---------------------------------------------------------------------------
def build_bass(split=True):
    nc = bass.Bass()
    pin = nc.dram_tensor("pin", [XS, CIN], BF16, kind="ExternalInput").ap()
    psc = nc.dram_tensor("psc", [XS, NSCAL], F32, kind="ExternalInput").ap()
    pout = nc.dram_tensor("pout", [XS, COUT], BF16, kind="ExternalOutput").ap()

    with tile.TileContext(nc) as tc:
        with tc.tile_pool(name="m", bufs=1) as pool, \
             tc.tile_pool(name="p", bufs=1, space="PSUM") as ppool:
            scal = pool.tile([XS, NSCAL], F32, tag="scal")
            ident = pool.tile([XS, 128], BF16, tag="ident")
            diags = pool.tile([XS, NDIAG * 128], BF16, tag="diags")

            # issue order doubles as queue priority: scal, ident, rows 1..4
            nc.sync.dma_start(scal[:], psc[:])
            nc.sync.dma_start(ident[:], pin[:, 0:128])

            row_in = {}
            for il in range(1, L1):
                ns = il + 1
                t = pool.tile([XS, (2 * ns + 2) * NV], BF16,
                              name=f"in{il}", tag=f"in{il}")
                o = IN_OFF[il]
                nc.sync.dma_start(t[:], pin[:, o:o + (2 * ns + 2) * NV])
                row_in[il] = t

            # on-device diagonal weights: diag(col c) = ident * scal[:, c]
            for i, ckey in enumerate(DIAG_ORDER):
                dst = diags[:, i * 128:(i + 1) * 128]
                sc = scal[:, COL[ckey]:COL[ckey] + 1]
                if i < NDIAG_DVE:
                    nc.vector.tensor_scalar_mul(dst, ident[:], sc)
                else:
                    nc.scalar.mul(dst, ident[:], sc)

            # G tiles for rows 3-4 inner slots: G = F[im-1] + 2cB*F[im+1]
            # feed (DVE, imm scalar): P = 2cB * F[im+1]; add (Pool): G = P+F
            g_tile = {}
            p_tile = {}
            for il in G_ROWS:
                ni = il - 1
                g_tile[il] = pool.tile([XS, 2 * ni * NV], BF16,
                                       name=f"g{il}", tag=f"g{il}")
                p_tile[il] = pool.tile([XS, 2 * ni * NV], BF16,
                                       name=f"p{il}", tag=f"p{il}")
            for (il, im) in G_PAIRS:
                ni = il - 1
                ns = il + 1
                t = row_in[il]
                S = ns * NV
                Sg = ni * NV

                def fslot(k):
                    return t[:, k * NV:(k + 1) * NV]

                pslot = p_tile[il][:, (im - 1) * NV:im * NV]
                gslot = g_tile[il][:, (im - 1) * NV:im * NV]
                nc.vector.tensor_scalar_mul(
                    _pair(pslot, Sg), _pair(fslot(im + 1), S),
                    2.0 * _cB(il, im))
                nc.gpsimd.tensor_tensor(
                    _pair(gslot, Sg), _pair(pslot, Sg),
                    _pair(fslot(im - 1), S), ADD)

            # PE warm-up: ramp the pstate before row-1 data arrives
            wbank = ppool.tile([XS, NV], F32, name="wbank", tag="wbank")
            for _ in range(20):
                nc.tensor.matmul(wbank[:], ident[:], diags[:, 0:NV],
                                 start=True, stop=True)

            def dg(ckey):
                i = DIAG_POS[ckey]
                return diags[:, i * 128:(i + 1) * 128]

            def rhs_ap(il, ref):
                kind, k = ref
                if kind in ("gr", "gi"):
                    ni = il - 1
                    s = (k - 1) if kind == "gr" else (ni + k - 1)
                    return g_tile[il][:, s * NV:(s + 1) * NV]
                t = row_in[il]
                ns = il + 1
                base = {"fr": 0, "fi": ns, "d0r": 2 * ns, "d0i": 2 * ns + 1}
                s = base[kind] + k
                return t[:, s * NV:(s + 1) * NV]

            row_out = {}
            for il in range(1, L1):
                ns = il + 1
                row_out[il] = pool.tile([XS, 2 * ns * NV], BF16,
                                        name=f"out{il}", tag=f"out{il}")

            def out_ap(il, kind, im):
                # pair-interleaved: slot index = 2*im + (0 if dr else 1)
                s = 2 * im + (0 if kind.startswith("dr") else 1)
                return row_out[il][:, s * NV:(s + 1) * NV]

            banks = [ppool.tile([XS, NV], F32, name=f"bank{i}",
                                tag=f"bank{i}") for i in range(7)]

            # emit in waves of 7 slots; group matmuls by diagonal in a wave
            for w0 in range(0, len(SLOTS), 7):
                wave = SLOTS[w0:w0 + 7]
                mms = []          # (diag_pos, bank_idx, ckey, il, ref)
                plans = []
                for j, (il, kind, im) in enumerate(wave):
                    pe, fold = _pe_plan(il, kind, im)
                    plans.append((il, kind, im, fold))
                    for ckey, ref in pe:
                        mms.append((DIAG_POS[ckey], j, ckey, il, ref))
                mms.sort(key=lambda m: m[0])
                remaining = [sum(1 for m in mms if m[1] == j)
                             for j in range(len(wave))]
                started = [False] * len(wave)
                for _, j, ckey, il, ref in mms:
                    remaining[j] -= 1
                    nc.tensor.matmul(banks[j][:], dg(ckey), rhs_ap(il, ref),
                                     start=not started[j],
                                     stop=remaining[j] == 0)
                    started[j] = True
                # drains + pair DMAs in slot order
                for j, (il, kind, im, fold) in enumerate(plans):
                    dst = out_ap(il, kind, im)
                    if fold is not None:
                        ckey, ref = fold
                        c = COL[ckey]
                        nc.vector.scalar_tensor_tensor(
                            dst, rhs_ap(il, ref), scal[:, c:c + 1],
                            banks[j][:], MULT, ADD)
                    else:
                        nc.scalar.copy(dst, banks[j][:])
                    if kind in ("di0", "di"):
                        o = OUT_OFF[il] + 2 * im * NV
                        nc.sync.dma_start(
                            pout[:, o:o + 2 * NV],
                            row_out[il][:, 2 * im * NV:(2 * im + 2) * NV])

    if split:
        split_multiwaits(nc)
    return nc


# ---------------------------------------------------------------------------
def pack_inputs(prev_f_re, prev_f_im, delta0_re, delta0_im, b):
    """-> per-core {'pin': [XS, CIN] bf16, 'psc': [XS, NSCAL] f32}."""
    in_maps = []
    for c in range(NCORES):
        X = slice(c * XS, (c + 1) * XS)
        b_sh = np.asarray(b[X], np.float32)
        p = np.zeros((XS, CIN), NPBF16)
        p[:, 0:128] = np.eye(XS, dtype=NPBF16)
        for il in range(1, L1):
            o = IN_OFF[il]
            ns = il + 1
            p[:, o:o + ns * NV] = (
                np.asarray(prev_f_re[il, :ns, X, :], np.float32)
                .transpose(1, 0, 2).reshape(XS, ns * NV).astype(NPBF16))
            o += ns * NV
            p[:, o:o + ns * NV] = (
                np.asarray(prev_f_im[il, :ns, X, :], np.float32)
                .transpose(1, 0, 2).reshape(XS, ns * NV).astype(NPBF16))
            o += ns * NV
            p[:, o:o + NV] = np.asarray(
                delta0_re[il, 0, X, :], np.float32).astype(NPBF16)
            o += NV
            p[:, o:o + NV] = np.asarray(
                delta0_im[il, 0, X, :], np.float32).astype(NPBF16)
        in_maps.append({"pin": p, "psc": build_scal(b_sh)})
    return in_maps


def unpack_outputs(results, delta0_re, delta0_im):
    out = np.zeros((L1, L1, NX, NV), np.complex64)
    out[0, 0] = np.asarray(delta0_re[0, 0]) + 1j * np.asarray(delta0_im[0, 0])
    for c in range(NCORES):
        X = slice(c * XS, (c + 1) * XS)
        p = results[c]["pout"].astype(np.float32)
        for il in range(1, L1):
            o = OUT_OFF[il]
            ns = il + 1
            blk = p[:, o:o + 2 * ns * NV].reshape(XS, ns, 2, NV)
            dr = blk[:, :, 0, :].transpose(1, 0, 2)
            di = blk[:, :, 1, :].transpose(1, 0, 2)
            out[il, :ns, X, :] = dr + 1j * di
    return out


_NC_CACHE = None


def get_nc():
    global _NC_CACHE
    if _NC_CACHE is None:
        _NC_CACHE = build_bass()
    return _NC_CACHE


def kernel(prev_f_re, prev_f_im, delta0_re, delta0_im, b, v):
    in_maps = pack_inputs(prev_f_re, prev_f_im, delta0_re, delta0_im, b)
    res = run_bass_kernel_spmd(get_nc(), in_maps, list(range(NCORES)))
    return unpack_outputs(res.results, delta0_re, delta0_im)


# revision 9
# speedup vs baseline: 2.0786x; 1.0266x over previous
"""Trainium2 Bass kernel for nn_Bdfdv_51170240364850 (gnn_message_passing).

Computes, for mode pairs (il, im) with im <= il (L1 = 5 modes each way) and
grid (nx=1024, nv=512):

  D[il,im] = base + (-1j)*im*bx*F[il,im] + cB*bm*F[il,im+1]
             + [im==0] Re(cC*bp*F[il,1])
  base     = 0.5*bm*F[il,im-1]  (il>=1, 1<=im<=il)   else  D0[il,im]

with bx = b[:,0], bm = b[:,1]+1j b[:,2], bp = conj(bm),
cB = -(il-im)(il+im+1)/2, cC = -il(il+1).

Strategy: pure data-parallel over nx across 8 NeuronCores (nx=128 per core on
the 128 SBUF partitions), bf16 end-to-end (tolerance 2e-2; bf16 keeps
~7e-3). Per-x scalar products run on the Tensor engine as diagonal-weight
matmuls accumulating in PSUM:

    out[x, :] += c(x) * in[x, :]   ==   PSUM += diag(c).T @ in

Each of the 28 valid output slots (dr/di per (il, im), row 0 on host) is one
PSUM-bank chain of 2-5 matmuls (N=512). Work split that keeps every engine
busy:

 - PE: diagonal matmuls (74 + warm-up), diagonals built on-device from an
   identity tile x f32 coefficient columns (DVE/ACT tensor_scalar, ~200ns).
 - DVE: diag builds, G-feed tensor_scalar ops, and scalar_tensor_tensor
   drains for rows 3-4 that FOLD one product (b0/q2 term) into the
   PSUM->SBUF drain, cutting PE work.
 - ACT: remaining diag builds + plain copy drains (rows 1-2, row 3 im=0 and
   edge).
 - Pool (GpSimd): builds G = F[im-1] + 2cB*F[im+1] (constant integer
   coefficients) for rows 3-4 inner slots via tensor_tensor adds, so each
   inner slot needs only 2 matmuls (h1*G, -/+h2*G~) instead of 4.

Matmuls are emitted in waves of 7 PSUM banks, grouped by diagonal within a
wave to maximize stationary-weight reuse. Output slots are pair-interleaved
and DMA out per (dr,di) pair as soon as drained.
"""

import numpy as np
import ml_dtypes

import bass_rust
import concourse.bass as bass
import concourse.tile as tile
from concourse import mybir
from concourse.bass_utils import run_bass_kernel_spmd

L1 = 5
NX = 1024
NV = 512
NCORES = 8
XS = NX // NCORES  # 128 = SBUF partitions

F32 = mybir.dt.float32
BF16 = mybir.dt.bfloat16
NPBF16 = ml_dtypes.bfloat16

MULT = mybir.AluOpType.mult
ADD = mybir.AluOpType.add

# ---------------------------------------------------------------------------
# coefficient registry
PAIRS = [(2, 1), (3, 1), (3, 2), (4, 1), (4, 2), (4, 3)]  # inner (il, im)


def _cB(il, im):
    return -(il - im) * (il + im + 1) / 2.0


COL = {"one": 0, "h1": 1, "h2p": 2, "h2n": 3}
for _il in range(1, L1):
    COL[("q1", _il)] = 3 + _il           # 4..7
    COL[("r1", _il)] = 7 + _il           # 8..11
    COL[("q2", _il)] = 11 + _il          # 12..15
for _m in range(1, L1):
    COL[("b0p", _m)] = 15 + _m           # 16..19
    COL[("b0n", _m)] = 19 + _m           # 20..23
for _k, _p in enumerate(PAIRS):
    COL[("c1",) + _p] = 24 + 3 * _k
    COL[("c2p",) + _p] = 25 + 3 * _k
    COL[("c2n",) + _p] = 26 + 3 * _k
NSCAL = 48  # padded f32 table width


def coeff_vec(key, b0, b1, b2):
    if key == "one":
        return np.ones_like(b0)
    if key == "h1":
        return 0.5 * b1
    if key == "h2p":
        return 0.5 * b2
    if key == "h2n":
        return -0.5 * b2
    tag = key[0]
    if tag == "q1":
        return 3.0 * _cB(key[1], 0) * b1
    if tag == "r1":
        return _cB(key[1], 0) * b1
    if tag == "q2":
        return _cB(key[1], 0) * b2
    if tag == "b0p":
        return key[1] * b0
    if tag == "b0n":
        return -key[1] * b0
    cB = _cB(key[1], key[2])
    if tag == "c1":
        return cB * b1
    if tag == "c2p":
        return cB * b2
    if tag == "c2n":
        return -cB * b2
    raise ValueError(key)


def build_scal(b_sh):
    """b_sh: [XS, 3] f32 -> [XS, NSCAL] f32 (STT-drain scalars by COL)."""
    b0, b1, b2 = b_sh[:, 0], b_sh[:, 1], b_sh[:, 2]
    s = np.zeros((XS, NSCAL), np.float32)
    for key, c in COL.items():
        s[:, c] = coeff_vec(key, b0, b1, b2)
    return s


# ---------------------------------------------------------------------------
# slot schedule: which products run on PE, which fold into the DVE drain,
# and which inner slots read the Pool-built G tiles.
SLOTS = []
for _il in range(1, L1):
    SLOTS.append((_il, "dr0", 0))
    SLOTS.append((_il, "di0", 0))
    for _im in range(1, _il + 1):
        SLOTS.append((_il, "dr", _im))
        SLOTS.append((_il, "di", _im))

G_ROWS = (2, 3, 4)       # rows whose inner slots use G tiles
G_PAIRS = [(il, im) for il in G_ROWS for im in range(1, il)]
# G adds on Pool except these (DVE tensor_tensor is ~3x faster; used where
# Pool would finish too late or DVE has early slack)
G_TT_DVE = {(2, 1), (4, 3)}


def _is_kill(il, kind, im):
    """STT-drain (DVE) slots: fold the b0/q2 product into the drain."""
    if il == 3 and kind in ("dr", "di") and im < il:
        return True
    return il == 4 and not (kind == "dr" and im >= 3)


def _pe_plan(il, kind, im):
    """-> (pe_products, fold or None); products are (col_key, rhs_ref),
    rhs_ref = (space, k) with space in fr/fi/gr/gi/d0r/d0i."""
    kill = _is_kill(il, kind, im)
    if kind == "dr0":
        pe = [("one", ("d0r", 0)), (("q1", il), ("fr", 1))]
        last = (("q2", il), ("fi", 1))
    elif kind == "di0":
        pe = [("one", ("d0i", 0)), (("r1", il), ("fi", 1))]
        last = (("q2", il), ("fr", 1))
    elif kind == "dr":
        if im < il and il in G_ROWS:
            pe = [("h1", ("gr", im)), ("h2n", ("gi", im))]
        elif im < il:
            pe = [("h1", ("fr", im - 1)), ("h2n", ("fi", im - 1)),
                  (("c1", il, im), ("fr", im + 1)),
                  (("c2n", il, im), ("fi", im + 1))]
        else:
            pe = [("h1", ("fr", im - 1)), ("h2n", ("fi", im - 1))]
        last = ((("b0p", im)), ("fi", im))
    elif kind == "di":
        if im < il and il in G_ROWS:
            pe = [("h1", ("gi", im)), ("h2p", ("gr", im))]
        elif im < il:
            pe = [("h1", ("fi", im - 1)), ("h2p", ("fr", im - 1)),
                  (("c1", il, im), ("fi", im + 1)),
                  (("c2p", il, im), ("fr", im + 1))]
        else:
            pe = [("h1", ("fi", im - 1)), ("h2p", ("fr", im - 1))]
        last = ((("b0n", im)), ("fr", im))
    else:
        raise ValueError(kind)
    if kill:
        return pe, last
    return pe + [last], None


# diag keys used on PE, in first-use order
DIAG_ORDER = []
for _s in SLOTS:
    for _ck, _ in _pe_plan(*_s)[0]:
        if _ck not in DIAG_ORDER:
            DIAG_ORDER.append(_ck)
NDIAG = len(DIAG_ORDER)
DIAG_POS = {k: i for i, k in enumerate(DIAG_ORDER)}
NDIAG_DVE = 12           # first N built on DVE, rest on ACT

# packed input layout (bf16): [identity (128 cols) | row blocks il=1..4]
# row block: fr slots (ns), fi slots (ns), d0r, d0i  -- each slot NV cols
IN_OFF = {}
_o = 128
for _il in range(1, L1):
    IN_OFF[_il] = _o
    _o += (2 * (_il + 1) + 2) * NV
CIN = _o

# packed output layout (bf16): row blocks il=1..4, pair-interleaved:
# [dr0, di0, dr1, di1, ...]
OUT_OFF = {}
_o = 0
for _il in range(1, L1):
    OUT_OFF[_il] = _o
    _o += 2 * (_il + 1) * NV
COUT = _o


# ---------------------------------------------------------------------------
# walrus in this container rejects >1 sync-wait per instruction; hoist
# extras onto same-engine NOPs.
def split_multiwaits(nc):
    for f in nc.m.functions:
        for blk in f.blocks:
            new = []
            changed = False
            for ins in blk.instructions:
                si = ins.sync_info
                if si is not None and len(si.on_wait) > 1:
                    waits = list(si.on_wait)
                    for w in waits[:-1]:
                        nop = mybir.InstNoOp(
                            name=nc.get_next_instruction_name(),
                            engine=ins.engine,
                            bass_nofuse=True,
                            sync_info=mybir.SyncInfo(on_wait=[w],
                                                     on_update=[]),
                        )
                        new.append(nop)
                    ins.sync_info = bass_rust.SyncInfo(
                        on_wait=[waits[-1]], on_update=list(si.on_update))
                    changed = True
                new.append(ins)
            if changed:
                blk.instructions = new


def _pair(ap, step_elems, nblocks=2):
    """Contiguous [P, L] AP -> [P, nblocks, L] with element step between
    blocks."""
    c = ap.copy()
    v = c.ap
    last = v.pop()
    v.append((step_elems, nblocks))
    v.append(tuple(last))
    c.ap = v
    return c


# ---------------------------------------------------------------------------
def build_bass(split=True):
    nc = bass.Bass()
    pin = nc.dram_tensor("pin", [XS, CIN], BF16, kind="ExternalInput").ap()
    psc = nc.dram_tensor("psc", [XS, NSCAL], F32, kind="ExternalInput").ap()
    pout = nc.dram_tensor("pout", [XS, COUT], BF16, kind="ExternalOutput").ap()

    with tile.TileContext(nc) as tc:
        with tc.tile_pool(name="m", bufs=1) as pool, \
             tc.tile_pool(name="p", bufs=1, space="PSUM") as ppool:
            scal = pool.tile([XS, NSCAL], F32, tag="scal")
            ident = pool.tile([XS, 128], BF16, tag="ident")
            diags = pool.tile([XS, NDIAG * 128], BF16, tag="diags")

            # issue order doubles as queue priority: scal, ident, rows 1..4
            nc.sync.dma_start(scal[:], psc[:])
            nc.sync.dma_start(ident[:], pin[:, 0:128])

            row_in = {}
            for il in range(1, L1):
                ns = il + 1
                t = pool.tile([XS, (2 * ns + 2) * NV], BF16,
                              name=f"in{il}", tag=f"in{il}")
                o = IN_OFF[il]
                if il == 1:
                    # priority part [fr1, fi1, d0r, d0i] lands first so the
                    # im=0 chains can start ~1.5us earlier
                    nc.sync.dma_start(t[:, 0:4 * NV], pin[:, o:o + 4 * NV])
                    nc.sync.dma_start(t[:, 4 * NV:6 * NV],
                                      pin[:, o + 4 * NV:o + 6 * NV])
                else:
                    nc.sync.dma_start(t[:],
                                      pin[:, o:o + (2 * ns + 2) * NV])
                row_in[il] = t

            # on-device diagonal weights: diag(col c) = ident * scal[:, c]
            for i, ckey in enumerate(DIAG_ORDER):
                dst = diags[:, i * 128:(i + 1) * 128]
                sc = scal[:, COL[ckey]:COL[ckey] + 1]
                if i < NDIAG_DVE:
                    nc.vector.tensor_scalar_mul(dst, ident[:], sc)
                else:
                    nc.scalar.mul(dst, ident[:], sc)

            # G tiles for rows 3-4 inner slots: G = F[im-1] + 2cB*F[im+1]
            # feed (DVE, imm scalar): P = 2cB * F[im+1]; add (Pool): G = P+F
            g_tile = {}
            p_tile = {}
            for il in G_ROWS:
                ni = il - 1
                g_tile[il] = pool.tile([XS, 2 * ni * NV], BF16,
                                       name=f"g{il}", tag=f"g{il}")
                p_tile[il] = pool.tile([XS, 2 * ni * NV], BF16,
                                       name=f"p{il}", tag=f"p{il}")
            for (il, im) in G_PAIRS:
                ni = il - 1
                ns = il + 1
                t = row_in[il]
                S = ns * NV
                Sg = ni * NV

                def fslot(k):
                    return t[:, k * NV:(k + 1) * NV]

                pslot = p_tile[il][:, (im - 1) * NV:im * NV]
                gslot = g_tile[il][:, (im - 1) * NV:im * NV]
                nc.vector.tensor_scalar_mul(
                    _pair(pslot, Sg), _pair(fslot(im + 1), S),
                    2.0 * _cB(il, im))
                eng = nc.vector if (il, im) in G_TT_DVE else nc.gpsimd
                eng.tensor_tensor(
                    _pair(gslot, Sg), _pair(pslot, Sg),
                    _pair(fslot(im - 1), S), ADD)

            # PE warm-up: ramp the pstate before row-1 data arrives.
            # scratch is never written: the values are irrelevant.
            scratch = pool.tile([XS, NV], BF16, tag="scratch")
            nc.gpsimd.memset(scratch[:], 0)
            wbank = ppool.tile([XS, NV], F32, name="wbank", tag="wbank")
            for _ in range(20):
                nc.tensor.matmul(wbank[:], ident[:], scratch[:],
                                 start=True, stop=True)

            def dg(ckey):
                i = DIAG_POS[ckey]
                return diags[:, i * 128:(i + 1) * 128]

            ROW1_SLOT = {("fr", 1): 0, ("fi", 1): 1, ("d0r", 0): 2,
                         ("d0i", 0): 3, ("fr", 0): 4, ("fi", 0): 5}

            def rhs_ap(il, ref):
                kind, k = ref
                if kind in ("gr", "gi"):
                    ni = il - 1
                    s = (k - 1) if kind == "gr" else (ni + k - 1)
                    return g_tile[il][:, s * NV:(s + 1) * NV]
                t = row_in[il]
                if il == 1:
                    s = ROW1_SLOT[(kind, k)]
                else:
                    ns = il + 1
                    base = {"fr": 0, "fi": ns,
                            "d0r": 2 * ns, "d0i": 2 * ns + 1}
                    s = base[kind] + k
                return t[:, s * NV:(s + 1) * NV]

            row_out = {}
            for il in range(1, L1):
                ns = il + 1
                row_out[il] = pool.tile([XS, 2 * ns * NV], BF16,
                                        name=f"out{il}", tag=f"out{il}")

            def out_ap(il, kind, im):
                # pair-interleaved: slot index = 2*im + (0 if dr else 1)
                s = 2 * im + (0 if kind.startswith("dr") else 1)
                return row_out[il][:, s * NV:(s + 1) * NV]

            banks = [ppool.tile([XS, NV], F32, name=f"bank{i}",
                                tag=f"bank{i}") for i in range(7)]

            # emit in waves of 7 slots; group matmuls by diagonal in a wave
            for w0 in range(0, len(SLOTS), 7):
                wave = SLOTS[w0:w0 + 7]
                mms = []          # (diag_pos, bank_idx, ckey, il, ref)
                plans = []
                for j, (il, kind, im) in enumerate(wave):
                    pe, fold = _pe_plan(il, kind, im)
                    plans.append((il, kind, im, fold))
                    for ckey, ref in pe:
                        mms.append((DIAG_POS[ckey], j, ckey, il, ref))
                mms.sort(key=lambda m: m[0])
                remaining = [sum(1 for m in mms if m[1] == j)
                             for j in range(len(wave))]
                started = [False] * len(wave)
                for _, j, ckey, il, ref in mms:
                    remaining[j] -= 1
                    nc.tensor.matmul(banks[j][:], dg(ckey), rhs_ap(il, ref),
                                     start=not started[j],
                                     stop=remaining[j] == 0)
                    started[j] = True
                # drains + pair DMAs in slot order
                for j, (il, kind, im, fold) in enumerate(plans):
                    dst = out_ap(il, kind, im)
                    if fold is not None:
                        ckey, ref = fold
                        c = COL[ckey]
                        nc.vector.scalar_tensor_tensor(
                            dst, rhs_ap(il, ref), scal[:, c:c + 1],
                            banks[j][:], MULT, ADD)
                    else:
                        nc.scalar.copy(dst, banks[j][:])
                    if kind in ("di0", "di"):
                        o = OUT_OFF[il] + 2 * im * NV
                        nc.sync.dma_start(
                            pout[:, o:o + 2 * NV],
                            row_out[il][:, 2 * im * NV:(2 * im + 2) * NV])

    if split:
        split_multiwaits(nc)
    return nc


# ---------------------------------------------------------------------------
def pack_inputs(prev_f_re, prev_f_im, delta0_re, delta0_im, b):
    """-> per-core {'pin': [XS, CIN] bf16, 'psc': [XS, NSCAL] f32}."""
    in_maps = []
    for c in range(NCORES):
        X = slice(c * XS, (c + 1) * XS)
        b_sh = np.asarray(b[X], np.float32)
        p = np.zeros((XS, CIN), NPBF16)
        p[:, 0:128] = np.eye(XS, dtype=NPBF16)
        def bf(a):
            return np.asarray(a, np.float32).astype(NPBF16)

        # row 1 slot order matches ROW1_SLOT in build_bass
        o = IN_OFF[1]
        for i, a in enumerate([prev_f_re[1, 1, X], prev_f_im[1, 1, X],
                               delta0_re[1, 0, X], delta0_im[1, 0, X],
                               prev_f_re[1, 0, X], prev_f_im[1, 0, X]]):
            p[:, o + i * NV:o + (i + 1) * NV] = bf(a)
        for il in range(2, L1):
            o = IN_OFF[il]
            ns = il + 1
            p[:, o:o + ns * NV] = (
                np.asarray(prev_f_re[il, :ns, X, :], np.float32)
                .transpose(1, 0, 2).reshape(XS, ns * NV).astype(NPBF16))
            o += ns * NV
            p[:, o:o + ns * NV] = (
                np.asarray(prev_f_im[il, :ns, X, :], np.float32)
                .transpose(1, 0, 2).reshape(XS, ns * NV).astype(NPBF16))
            o += ns * NV
            p[:, o:o + NV] = np.asarray(
                delta0_re[il, 0, X, :], np.float32).astype(NPBF16)
            o += NV
            p[:, o:o + NV] = np.asarray(
                delta0_im[il, 0, X, :], np.float32).astype(NPBF16)
        in_maps.append({"pin": p, "psc": build_scal(b_sh)})
    return in_maps


def unpack_outputs(results, delta0_re, delta0_im):
    out = np.zeros((L1, L1, NX, NV), np.complex64)
    out[0, 0] = np.asarray(delta0_re[0, 0]) + 1j * np.asarray(delta0_im[0, 0])
    for c in range(NCORES):
        X = slice(c * XS, (c + 1) * XS)
        p = results[c]["pout"].astype(np.float32)
        for il in range(1, L1):
            o = OUT_OFF[il]
            ns = il + 1
            blk = p[:, o:o + 2 * ns * NV].reshape(XS, ns, 2, NV)
            dr = blk[:, :, 0, :].transpose(1, 0, 2)
            di = blk[:, :, 1, :].transpose(1, 0, 2)
            out[il, :ns, X, :] = dr + 1j * di
    return out


_NC_CACHE = None


def get_nc():
    global _NC_CACHE
    if _NC_CACHE is None:
        _NC_CACHE = build_bass()
    return _NC_CACHE


def kernel(prev_f_re, prev_f_im, delta0_re, delta0_im, b, v):
    in_maps = pack_inputs(prev_f_re, prev_f_im, delta0_re, delta0_im, b)
    res = run_bass_kernel_spmd(get_nc(), in_maps, list(range(NCORES)))
    return unpack_outputs(res.results, delta0_re, delta0_im)


# revision 10
# speedup vs baseline: 2.2421x; 1.0787x over previous
"""Trainium2 Bass kernel for nn_Bdfdv_51170240364850 (gnn_message_passing).

Computes, for mode pairs (il, im) with im <= il (L1 = 5 modes each way) and
grid (nx=1024, nv=512):

  D[il,im] = base + (-1j)*im*bx*F[il,im] + cB*bm*F[il,im+1]
             + [im==0] Re(cC*bp*F[il,1])
  base     = 0.5*bm*F[il,im-1]  (il>=1, 1<=im<=il)   else  D0[il,im]

with bx = b[:,0], bm = b[:,1]+1j b[:,2], bp = conj(bm),
cB = -(il-im)(il+im+1)/2, cC = -il(il+1).

Strategy: pure data-parallel over nx across 8 NeuronCores (nx=128 per core on
the 128 SBUF partitions), bf16 end-to-end (tolerance 2e-2; bf16 keeps
~7e-3). Per-x scalar products run on the Tensor engine as diagonal-weight
matmuls accumulating in PSUM:

    out[x, :] += c(x) * in[x, :]   ==   PSUM += diag(c).T @ in

Each of the 28 valid output slots (dr/di per (il, im), row 0 on host) is one
PSUM-bank chain of 2-5 matmuls (N=512). Work split that keeps every engine
busy:

 - PE: diagonal matmuls (74 + warm-up), diagonals built on-device from an
   identity tile x f32 coefficient columns (DVE/ACT tensor_scalar, ~200ns).
 - DVE: diag builds, G-feed tensor_scalar ops, and scalar_tensor_tensor
   drains for rows 3-4 that FOLD one product (b0/q2 term) into the
   PSUM->SBUF drain, cutting PE work.
 - ACT: remaining diag builds + plain copy drains (rows 1-2, row 3 im=0 and
   edge).
 - Pool (GpSimd): builds G = F[im-1] + 2cB*F[im+1] (constant integer
   coefficients) for rows 3-4 inner slots via tensor_tensor adds, so each
   inner slot needs only 2 matmuls (h1*G, -/+h2*G~) instead of 4.

Matmuls are emitted in waves of 7 PSUM banks, grouped by diagonal within a
wave to maximize stationary-weight reuse. Output slots are pair-interleaved
and DMA out per (dr,di) pair as soon as drained.
"""

import numpy as np
import ml_dtypes

import bass_rust
import concourse.bass as bass
import concourse.tile as tile
from concourse import mybir
from concourse.bass_utils import run_bass_kernel_spmd

L1 = 5
NX = 1024
NV = 512
NCORES = 8
XS = NX // NCORES  # 128 = SBUF partitions

F32 = mybir.dt.float32
BF16 = mybir.dt.bfloat16
NPBF16 = ml_dtypes.bfloat16

MULT = mybir.AluOpType.mult
ADD = mybir.AluOpType.add

# ---------------------------------------------------------------------------
# coefficient registry
PAIRS = [(2, 1), (3, 1), (3, 2), (4, 1), (4, 2), (4, 3)]  # inner (il, im)


def _cB(il, im):
    return -(il - im) * (il + im + 1) / 2.0


COL = {"one": 0, "h1": 1, "h2p": 2, "h2n": 3}
for _il in range(1, L1):
    COL[("q1", _il)] = 3 + _il           # 4..7
    COL[("r1", _il)] = 7 + _il           # 8..11
    COL[("q2", _il)] = 11 + _il          # 12..15
for _m in range(1, L1):
    COL[("b0p", _m)] = 15 + _m           # 16..19
    COL[("b0n", _m)] = 19 + _m           # 20..23
for _k, _p in enumerate(PAIRS):
    COL[("c1",) + _p] = 24 + 3 * _k
    COL[("c2p",) + _p] = 25 + 3 * _k
    COL[("c2n",) + _p] = 26 + 3 * _k
NSCAL = 48  # padded f32 table width


def coeff_vec(key, b0, b1, b2):
    if key == "one":
        return np.ones_like(b0)
    if key == "h1":
        return 0.5 * b1
    if key == "h2p":
        return 0.5 * b2
    if key == "h2n":
        return -0.5 * b2
    tag = key[0]
    if tag == "q1":
        return 3.0 * _cB(key[1], 0) * b1
    if tag == "r1":
        return _cB(key[1], 0) * b1
    if tag == "q2":
        return _cB(key[1], 0) * b2
    if tag == "b0p":
        return key[1] * b0
    if tag == "b0n":
        return -key[1] * b0
    cB = _cB(key[1], key[2])
    if tag == "c1":
        return cB * b1
    if tag == "c2p":
        return cB * b2
    if tag == "c2n":
        return -cB * b2
    raise ValueError(key)


def build_scal(b_sh):
    """b_sh: [XS, 3] f32 -> [XS, NSCAL] f32 (STT-drain scalars by COL)."""
    b0, b1, b2 = b_sh[:, 0], b_sh[:, 1], b_sh[:, 2]
    s = np.zeros((XS, NSCAL), np.float32)
    for key, c in COL.items():
        s[:, c] = coeff_vec(key, b0, b1, b2)
    return s


# ---------------------------------------------------------------------------
# slot schedule: which products run on PE, which fold into the DVE drain,
# and which inner slots read the Pool-built G tiles.
SLOTS = []
for _il in range(1, L1):
    SLOTS.append((_il, "dr0", 0))
    SLOTS.append((_il, "di0", 0))
    for _im in range(1, _il + 1):
        SLOTS.append((_il, "dr", _im))
        SLOTS.append((_il, "di", _im))

G_ROWS = (2, 3, 4)       # rows whose inner slots use G tiles
G_PAIRS = [(il, im) for il in G_ROWS for im in range(1, il)]
# G adds on Pool except these (DVE tensor_tensor is ~3x faster; used where
# Pool would finish too late or DVE has early slack)
G_TT_DVE = {(2, 1), (3, 1), (3, 2), (4, 1), (4, 2), (4, 3)}


def _is_kill(il, kind, im):
    """STT-drain (DVE) slots: fold the b0/q2 product into the drain."""
    if il == 3 and kind in ("dr", "di") and im < il:
        return True
    return il == 4 and not (kind == "dr" and im >= 3)


def _pe_plan(il, kind, im):
    """-> (pe_products, fold or None); products are (col_key, rhs_ref),
    rhs_ref = (space, k) with space in fr/fi/gr/gi/d0r/d0i."""
    kill = _is_kill(il, kind, im)
    if kind == "dr0":
        pe = [("one", ("d0r", 0)), (("q1", il), ("fr", 1))]
        last = (("q2", il), ("fi", 1))
    elif kind == "di0":
        pe = [("one", ("d0i", 0)), (("r1", il), ("fi", 1))]
        last = (("q2", il), ("fr", 1))
    elif kind == "dr":
        if im < il and il in G_ROWS:
            pe = [("h1", ("gr", im)), ("h2n", ("gi", im))]
        elif im < il:
            pe = [("h1", ("fr", im - 1)), ("h2n", ("fi", im - 1)),
                  (("c1", il, im), ("fr", im + 1)),
                  (("c2n", il, im), ("fi", im + 1))]
        else:
            pe = [("h1", ("fr", im - 1)), ("h2n", ("fi", im - 1))]
        last = ((("b0p", im)), ("fi", im))
    elif kind == "di":
        if im < il and il in G_ROWS:
            pe = [("h1", ("gi", im)), ("h2p", ("gr", im))]
        elif im < il:
            pe = [("h1", ("fi", im - 1)), ("h2p", ("fr", im - 1)),
                  (("c1", il, im), ("fi", im + 1)),
                  (("c2p", il, im), ("fr", im + 1))]
        else:
            pe = [("h1", ("fi", im - 1)), ("h2p", ("fr", im - 1))]
        last = ((("b0n", im)), ("fr", im))
    else:
        raise ValueError(kind)
    if kill:
        return pe, last
    return pe + [last], None


# diag keys used on PE, in first-use order
DIAG_ORDER = []
for _s in SLOTS:
    for _ck, _ in _pe_plan(*_s)[0]:
        if _ck not in DIAG_ORDER:
            DIAG_ORDER.append(_ck)
NDIAG = len(DIAG_ORDER)
DIAG_POS = {k: i for i, k in enumerate(DIAG_ORDER)}
NDIAG_DVE = 14           # first N built on DVE, rest on ACT

# packed input layout (bf16): [identity (128 cols) | row blocks il=1..4]
# row block: fr slots (ns), fi slots (ns), d0r, d0i  -- each slot NV cols
IN_OFF = {}
_o = 128
for _il in range(1, L1):
    IN_OFF[_il] = _o
    _o += (2 * (_il + 1) + 2) * NV
CIN = _o

# packed output layout (bf16): row blocks il=1..4, pair-interleaved:
# [dr0, di0, dr1, di1, ...]
OUT_OFF = {}
_o = 0
for _il in range(1, L1):
    OUT_OFF[_il] = _o
    _o += 2 * (_il + 1) * NV
COUT = _o


# ---------------------------------------------------------------------------
# walrus in this container rejects >1 sync-wait per instruction; hoist
# extras onto same-engine NOPs.
def split_multiwaits(nc):
    for f in nc.m.functions:
        for blk in f.blocks:
            new = []
            changed = False
            for ins in blk.instructions:
                si = ins.sync_info
                if si is not None and len(si.on_wait) > 1:
                    waits = list(si.on_wait)
                    for w in waits[:-1]:
                        nop = mybir.InstNoOp(
                            name=nc.get_next_instruction_name(),
                            engine=ins.engine,
                            bass_nofuse=True,
                            sync_info=mybir.SyncInfo(on_wait=[w],
                                                     on_update=[]),
                        )
                        new.append(nop)
                    ins.sync_info = bass_rust.SyncInfo(
                        on_wait=[waits[-1]], on_update=list(si.on_update))
                    changed = True
                new.append(ins)
            if changed:
                blk.instructions = new


def _pair(ap, step_elems, nblocks=2):
    """Contiguous [P, L] AP -> [P, nblocks, L] with element step between
    blocks."""
    c = ap.copy()
    v = c.ap
    last = v.pop()
    v.append((step_elems, nblocks))
    v.append(tuple(last))
    c.ap = v
    return c


# ---------------------------------------------------------------------------
def build_bass(split=True):
    nc = bass.Bass()
    pin = nc.dram_tensor("pin", [XS, CIN], BF16, kind="ExternalInput").ap()
    psc = nc.dram_tensor("psc", [XS, NSCAL], F32, kind="ExternalInput").ap()
    pout = nc.dram_tensor("pout", [XS, COUT], BF16, kind="ExternalOutput").ap()

    with tile.TileContext(nc) as tc:
        with tc.tile_pool(name="m", bufs=1) as pool, \
             tc.tile_pool(name="p", bufs=1, space="PSUM") as ppool:
            scal = pool.tile([XS, NSCAL], F32, tag="scal")
            ident = pool.tile([XS, 128], BF16, tag="ident")
            diags = pool.tile([XS, NDIAG * 128], BF16, tag="diags")

            # issue order doubles as queue priority: scal, ident, rows 1..4
            nc.sync.dma_start(scal[:], psc[:])
            nc.sync.dma_start(ident[:], pin[:, 0:128])

            row_in = {}
            for il in range(1, L1):
                ns = il + 1
                t = pool.tile([XS, (2 * ns + 2) * NV], BF16,
                              name=f"in{il}", tag=f"in{il}")
                o = IN_OFF[il]
                if il == 1:
                    # priority part [fr1, fi1, d0r, d0i] lands first so the
                    # im=0 chains can start ~1.5us earlier
                    nc.sync.dma_start(t[:, 0:4 * NV], pin[:, o:o + 4 * NV])
                    nc.sync.dma_start(t[:, 4 * NV:6 * NV],
                                      pin[:, o + 4 * NV:o + 6 * NV])
                else:
                    nc.sync.dma_start(t[:],
                                      pin[:, o:o + (2 * ns + 2) * NV])
                row_in[il] = t

            # on-device diagonal weights: diag(col c) = ident * scal[:, c]
            for i, ckey in enumerate(DIAG_ORDER):
                dst = diags[:, i * 128:(i + 1) * 128]
                sc = scal[:, COL[ckey]:COL[ckey] + 1]
                if i < NDIAG_DVE:
                    nc.vector.tensor_scalar_mul(dst, ident[:], sc)
                else:
                    nc.scalar.mul(dst, ident[:], sc)

            # G tiles for rows 3-4 inner slots: G = F[im-1] + 2cB*F[im+1]
            # feed (DVE, imm scalar): P = 2cB * F[im+1]; add (Pool): G = P+F
            g_tile = {}
            p_tile = {}
            for il in G_ROWS:
                ni = il - 1
                g_tile[il] = pool.tile([XS, 2 * ni * NV], BF16,
                                       name=f"g{il}", tag=f"g{il}")
                p_tile[il] = pool.tile([XS, 2 * ni * NV], BF16,
                                       name=f"p{il}", tag=f"p{il}")
            for (il, im) in G_PAIRS:
                ni = il - 1
                ns = il + 1
                t = row_in[il]
                S = ns * NV
                Sg = ni * NV

                def fslot(k):
                    return t[:, k * NV:(k + 1) * NV]

                pslot = p_tile[il][:, (im - 1) * NV:im * NV]
                gslot = g_tile[il][:, (im - 1) * NV:im * NV]
                nc.vector.tensor_scalar_mul(
                    _pair(pslot, Sg), _pair(fslot(im + 1), S),
                    2.0 * _cB(il, im))
                eng = nc.vector if (il, im) in G_TT_DVE else nc.gpsimd
                eng.tensor_tensor(
                    _pair(gslot, Sg), _pair(pslot, Sg),
                    _pair(fslot(im - 1), S), ADD)

            # PE warm-up: ramp the pstate before row-1 data arrives.
            # scratch is never written: the values are irrelevant.
            scratch = pool.tile([XS, NV], BF16, tag="scratch")
            nc.gpsimd.memset(scratch[:], 0)
            wbank = ppool.tile([XS, NV], F32, name="wbank", tag="wbank")
            for _ in range(8):
                nc.tensor.matmul(wbank[:], scratch[:, 0:128], scratch[:],
                                 start=True, stop=True)

            def dg(ckey):
                i = DIAG_POS[ckey]
                return diags[:, i * 128:(i + 1) * 128]

            ROW1_SLOT = {("fr", 1): 0, ("fi", 1): 1, ("d0r", 0): 2,
                         ("d0i", 0): 3, ("fr", 0): 4, ("fi", 0): 5}

            def rhs_ap(il, ref):
                kind, k = ref
                if kind in ("gr", "gi"):
                    ni = il - 1
                    s = (k - 1) if kind == "gr" else (ni + k - 1)
                    return g_tile[il][:, s * NV:(s + 1) * NV]
                t = row_in[il]
                if il == 1:
                    s = ROW1_SLOT[(kind, k)]
                else:
                    ns = il + 1
                    base = {"fr": 0, "fi": ns,
                            "d0r": 2 * ns, "d0i": 2 * ns + 1}
                    s = base[kind] + k
                return t[:, s * NV:(s + 1) * NV]

            row_out = {}
            for il in range(1, L1):
                ns = il + 1
                row_out[il] = pool.tile([XS, 2 * ns * NV], BF16,
                                        name=f"out{il}", tag=f"out{il}")

            def out_ap(il, kind, im):
                # pair-interleaved: slot index = 2*im + (0 if dr else 1)
                s = 2 * im + (0 if kind.startswith("dr") else 1)
                return row_out[il][:, s * NV:(s + 1) * NV]

            banks = [ppool.tile([XS, NV], F32, name=f"bank{i}",
                                tag=f"bank{i}") for i in range(7)]

            # emit in waves of 7 slots; group matmuls by diagonal in a wave
            for w0 in range(0, len(SLOTS), 7):
                wave = SLOTS[w0:w0 + 7]
                mms = []          # (diag_pos, bank_idx, ckey, il, ref)
                plans = []
                for j, (il, kind, im) in enumerate(wave):
                    pe, fold = _pe_plan(il, kind, im)
                    plans.append((il, kind, im, fold))
                    for ckey, ref in pe:
                        mms.append((DIAG_POS[ckey], j, ckey, il, ref))
                mms.sort(key=lambda m: m[0])
                remaining = [sum(1 for m in mms if m[1] == j)
                             for j in range(len(wave))]
                started = [False] * len(wave)
                for _, j, ckey, il, ref in mms:
                    remaining[j] -= 1
                    nc.tensor.matmul(banks[j][:], dg(ckey), rhs_ap(il, ref),
                                     start=not started[j],
                                     stop=remaining[j] == 0)
                    started[j] = True
                # drains + pair DMAs in slot order
                for j, (il, kind, im, fold) in enumerate(plans):
                    dst = out_ap(il, kind, im)
                    if fold is not None:
                        ckey, ref = fold
                        c = COL[ckey]
                        nc.vector.scalar_tensor_tensor(
                            dst, rhs_ap(il, ref), scal[:, c:c + 1],
                            banks[j][:], MULT, ADD)
                    else:
                        nc.scalar.copy(dst, banks[j][:])
                    if kind in ("di0", "di"):
                        o = OUT_OFF[il] + 2 * im * NV
                        nc.sync.dma_start(
                            pout[:, o:o + 2 * NV],
                            row_out[il][:, 2 * im * NV:(2 * im + 2) * NV])

    if split:
        split_multiwaits(nc)
    return nc


# ---------------------------------------------------------------------------
def pack_inputs(prev_f_re, prev_f_im, delta0_re, delta0_im, b):
    """-> per-core {'pin': [XS, CIN] bf16, 'psc': [XS, NSCAL] f32}."""
    in_maps = []
    for c in range(NCORES):
        X = slice(c * XS, (c + 1) * XS)
        b_sh = np.asarray(b[X], np.float32)
        p = np.zeros((XS, CIN), NPBF16)
        p[:, 0:128] = np.eye(XS, dtype=NPBF16)
        def bf(a):
            return np.asarray(a, np.float32).astype(NPBF16)

        # row 1 slot order matches ROW1_SLOT in build_bass
        o = IN_OFF[1]
        for i, a in enumerate([prev_f_re[1, 1, X], prev_f_im[1, 1, X],
                               delta0_re[1, 0, X], delta0_im[1, 0, X],
                               prev_f_re[1, 0, X], prev_f_im[1, 0, X]]):
            p[:, o + i * NV:o + (i + 1) * NV] = bf(a)
        for il in range(2, L1):
            o = IN_OFF[il]
            ns = il + 1
            p[:, o:o + ns * NV] = (
                np.asarray(prev_f_re[il, :ns, X, :], np.float32)
                .transpose(1, 0, 2).reshape(XS, ns * NV).astype(NPBF16))
            o += ns * NV
            p[:, o:o + ns * NV] = (
                np.asarray(prev_f_im[il, :ns, X, :], np.float32)
                .transpose(1, 0, 2).reshape(XS, ns * NV).astype(NPBF16))
            o += ns * NV
            p[:, o:o + NV] = np.asarray(
                delta0_re[il, 0, X, :], np.float32).astype(NPBF16)
            o += NV
            p[:, o:o + NV] = np.asarray(
                delta0_im[il, 0, X, :], np.float32).astype(NPBF16)
        in_maps.append({"pin": p, "psc": build_scal(b_sh)})
    return in_maps


def unpack_outputs(results, delta0_re, delta0_im):
    out = np.zeros((L1, L1, NX, NV), np.complex64)
    out[0, 0] = np.asarray(delta0_re[0, 0]) + 1j * np.asarray(delta0_im[0, 0])
    for c in range(NCORES):
        X = slice(c * XS, (c + 1) * XS)
        p = results[c]["pout"].astype(np.float32)
        for il in range(1, L1):
            o = OUT_OFF[il]
            ns = il + 1
            blk = p[:, o:o + 2 * ns * NV].reshape(XS, ns, 2, NV)
            dr = blk[:, :, 0, :].transpose(1, 0, 2)
            di = blk[:, :, 1, :].transpose(1, 0, 2)
            out[il, :ns, X, :] = dr + 1j * di
    return out


_NC_CACHE = None


def get_nc():
    global _NC_CACHE
    if _NC_CACHE is None:
        _NC_CACHE = build_bass()
    return _NC_CACHE


def kernel(prev_f_re, prev_f_im, delta0_re, delta0_im, b, v):
    in_maps = pack_inputs(prev_f_re, prev_f_im, delta0_re, delta0_im, b)
    res = run_bass_kernel_spmd(get_nc(), in_maps, list(range(NCORES)))
    return unpack_outputs(res.results, delta0_re, delta0_im)
